# revision 17
# baseline (speedup 1.0000x reference)
"""Channel-attention (XCA) block on 8 trn2 NeuronCores, data-parallel over batch.

Per core: x (4096, 768) -> qkv -> per-head channel attention (96x96 scores over
l2-normalized q,k transposed to (Ch, N)) -> proj.  All big matmuls run in bf16
with fp32 PSUM accumulation; norms/softmax in fp32.
"""

import numpy as np
from contextlib import ExitStack

import bass_rust
import concourse.bass as bass
import concourse.tile as tile
from concourse import mybir
from concourse.masks import make_identity
from concourse.bass_utils import run_bass_kernel_spmd

F32 = mybir.dt.float32
BF = mybir.dt.bfloat16

P = 128          # partitions
N = 4096         # tokens per core (batch element)
C = 768          # channels
H = 8            # heads
CH = 96          # channels per head
KC = C // P      # 6 contraction chunks of 128
NB = N // P      # 32 token blocks of 128
N5 = N // 512    # 8 token blocks of 512
G = 4            # head groups
HPG = H // G     # 2 heads per group
GC = HPG * CH    # 192 qkv columns per group
EPS = 1e-12


def build_nc():
    nc = bass.Bass()

    x_d = nc.dram_tensor("x", [N, C], F32, kind="ExternalInput")
    wqkv_d = nc.dram_tensor("Wqkv", [C, 3 * C], F32, kind="ExternalInput")
    temp_d = nc.dram_tensor("temperature", [H], F32, kind="ExternalInput")
    wproj_d = nc.dram_tensor("Wproj", [C, C], F32, kind="ExternalInput")
    bproj_d = nc.dram_tensor("bproj", [C], F32, kind="ExternalInput")
    y_d = nc.dram_tensor("y", [N, C], F32, kind="ExternalOutput")

    with ExitStack() as ctx:
        tc = ctx.enter_context(tile.TileContext(nc))
        persist = ctx.enter_context(tc.tile_pool(name="persist", bufs=1))

        # persistent SBUF: xT[c%128, (c//128)*N + n] = x[n, c]  (bf16)
        xT = persist.tile([P, KC * N], BF)
        # Wqkv bf16: wq[c%128, (c//128)*2304 + j] = Wqkv[c, j]
        wq = persist.tile([P, KC * 3 * C], BF)
        # attention output, channel-major: ot[c%128, (c//128)*N + n] = O[n, c]
        ot = persist.tile([P, KC * N], BF)

        ident128 = persist.tile([P, P], F32)
        make_identity(nc, ident128)
        ident96 = persist.tile([CH, CH], BF)
        make_identity(nc, ident96)
        ones_col = persist.tile([P, 1], BF)      # norm-matmul lhsT (K=128, M=1)
        nc.vector.memset(ones_col, 1.0)
        ones_row = persist.tile([1, P], BF)      # bias-matmul lhsT (K=1, M=128)
        nc.vector.memset(ones_row, 1.0)
        one1 = persist.tile([1, 1], F32)         # row->col matmul rhs
        nc.vector.memset(one1, 1.0)
        ones96 = persist.tile([1, CH], F32)
        nc.vector.memset(ones96, 1.0)

        temp_sb = persist.tile([1, H], F32)
        nc.sync.dma_start(out=temp_sb, in_=temp_d.rearrange("(a h) -> a h", a=1))
        bstage = persist.tile([1, C], F32)
        nc.sync.dma_start(out=bstage, in_=bproj_d.rearrange("(a c) -> a c", a=1))
        bproj_bf = persist.tile([1, C], BF)
        nc.vector.tensor_copy(bproj_bf, bstage)

        # ---- Phase W: load Wqkv, cast to bf16 ----
        with tc.tile_pool(name="wstage", bufs=2) as wstage:
            for kc in range(KC):
                st = wstage.tile([P, 3 * C], F32, tag="wst")
                nc.sync.dma_start(out=st, in_=wqkv_d[kc * P:(kc + 1) * P, :])
                nc.vector.tensor_copy(wq[:, kc * 3 * C:(kc + 1) * 3 * C], st)

        # ---- Phase T: x -> xT (PE transpose, downcast on evict) ----
        with tc.tile_pool(name="xstage", bufs=3) as xstage, \
             tc.tile_pool(name="tps", bufs=4, space="PSUM") as tps:
            for nb in range(NB):
                xt_ = xstage.tile([P, C], F32, tag="x")
                nc.sync.dma_start(out=xt_, in_=x_d[nb * P:(nb + 1) * P, :])
                for kc in range(KC):
                    pt = tps.tile([P, P], F32, tag="t")
                    nc.tensor.transpose(pt, xt_[:, kc * P:(kc + 1) * P], ident128)
                    nc.vector.tensor_copy(
                        xT[:, kc * N + nb * P: kc * N + (nb + 1) * P], pt)

        # ---- head-group loop ----
        gctx = ctx.enter_context(ExitStack())
        qk_pool = gctx.enter_context(tc.tile_pool(name="qk", bufs=1))
        # qkps tile holds q|k side by side: (128, 384) f32 = 1536B -> 1 bank
        qkps = gctx.enter_context(tc.tile_pool(name="qkps", bufs=2, space="PSUM"))
        nrmps = gctx.enter_context(tc.tile_pool(name="nrmps", bufs=1, space="PSUM"))
        sqpool = gctx.enter_context(tc.tile_pool(name="sq", bufs=3))
        small = gctx.enter_context(tc.tile_pool(name="small", bufs=4))
        vt_pool = gctx.enter_context(tc.tile_pool(name="vt", bufs=2))
        # vT-gen and out matmul PSUM share one 3-slot pool (same tag)
        hps = gctx.enter_context(tc.tile_pool(name="hps", bufs=3, space="PSUM"))
        sps = gctx.enter_context(tc.tile_pool(name="sps", bufs=1, space="PSUM"))
        tinyps = gctx.enter_context(tc.tile_pool(name="tinyps", bufs=1, space="PSUM"))

        for g in range(G):
            # q_sb/k_sb: (128 n, nb*GC + hh*CH + c) bf16 for this group's heads
            q_sb = qk_pool.tile([P, NB * GC], BF, tag="q")
            k_sb = qk_pool.tile([P, NB * GC], BF, tag="k")
            nqk = nrmps.tile([1, 2 * GC], F32, tag="nqk")  # [q sumsq | k sumsq]
            nq_ps = nqk[0:1, 0:GC]
            nk_ps = nqk[0:1, GC:2 * GC]

            for nb in range(NB):
                qkp = qkps.tile([P, 2 * GC], F32, tag="qkp")
                qp = qkp[:, 0:GC]
                kp = qkp[:, GC:2 * GC]
                # qp|kp live in one PSUM bank: one accumulation group (start
                # pending-zeroes the whole bank, k region accumulates onto 0)
                for kc in range(KC):
                    lhsT = xT[:, kc * N + nb * P: kc * N + (nb + 1) * P]
                    nc.tensor.matmul(
                        qp, lhsT, wq[:, kc * 3 * C + g * GC: kc * 3 * C + (g + 1) * GC],
                        start=(kc == 0), stop=False)
                    nc.tensor.matmul(
                        kp, lhsT,
                        wq[:, kc * 3 * C + C + g * GC: kc * 3 * C + C + (g + 1) * GC],
                        start=False, stop=(kc == KC - 1))
                qs = q_sb[:, nb * GC:(nb + 1) * GC]
                ks = k_sb[:, nb * GC:(nb + 1) * GC]
                nc.vector.tensor_copy(qs, qp)
                nc.vector.tensor_copy(ks, kp)
                qsq = sqpool.tile([P, GC], BF, tag="qsq")
                ksq = sqpool.tile([P, GC], BF, tag="ksq")
                nc.vector.tensor_mul(qsq, qs, qs)
                nc.vector.tensor_mul(ksq, ks, ks)
                nc.tensor.matmul(nq_ps, ones_col, qsq,
                                 start=(nb == 0), stop=False)
                nc.tensor.matmul(nk_ps, ones_col, ksq,
                                 start=False, stop=(nb == NB - 1))

            # rinv rows: 1 / max(sqrt(sumsq), eps)  (fp32, 1-partition ops)
            rq_row = small.tile([1, GC], F32, tag="rqr")
            rk_row = small.tile([1, GC], F32, tag="rkr")
            nc.scalar.activation(rq_row, nq_ps, mybir.ActivationFunctionType.Sqrt)
            nc.scalar.activation(rk_row, nk_ps, mybir.ActivationFunctionType.Sqrt)
            nc.vector.tensor_scalar_max(rq_row, rq_row, EPS)
            nc.vector.tensor_scalar_max(rk_row, rk_row, EPS)
            nc.vector.reciprocal(rq_row, rq_row)
            nc.vector.reciprocal(rk_row, rk_row)

            for hh in range(HPG):
                h = g * HPG + hh

                # rinvq column (96,1) via K=1 matmul: rq_col = rq_row_h^T @ [1]
                rq_ps = tinyps.tile([CH, 1], F32, tag="tp")
                nc.tensor.matmul(rq_ps, rq_row[0:1, hh * CH:(hh + 1) * CH], one1,
                                 start=True, stop=True)
                rq_col = small.tile([CH, 1], F32, tag="rqc")
                nc.vector.tensor_copy(rq_col, rq_ps)

                # temp broadcast row, then R = tempb^T @ rk_row_h (rank-1, 96x96)
                tempb = small.tile([1, CH], F32, tag="tb")
                nc.scalar.activation(tempb, ones96,
                                     mybir.ActivationFunctionType.Copy,
                                     scale=temp_sb[0:1, h:h + 1])
                r_ps = tinyps.tile([CH, CH], F32, tag="tp")
                nc.tensor.matmul(r_ps, tempb, rk_row[0:1, hh * CH:(hh + 1) * CH],
                                 start=True, stop=True)

                # vT for this head: (96 d, 4096 n) bf16, from Wqkv v-cols and xT
                vt_sb = vt_pool.tile([CH, N], BF, tag="vt")
                for n5 in range(N5):
                    vp = hps.tile([CH, 512], F32, tag="hp")
                    for kc in range(KC):
                        nc.tensor.matmul(
                            vp,
                            wq[:, kc * 3 * C + 2 * C + h * CH: kc * 3 * C + 2 * C + (h + 1) * CH],
                            xT[:, kc * N + n5 * 512: kc * N + (n5 + 1) * 512],
                            start=(kc == 0), stop=(kc == KC - 1))
                    nc.vector.tensor_copy(vt_sb[:, n5 * 512:(n5 + 1) * 512], vp)

                # raw scores S (96 c, 96 d), contraction over all 4096 n
                s_ps = sps.tile([CH, CH], F32, tag="s")
                for nb in range(NB):
                    nc.tensor.matmul(
                        s_ps,
                        q_sb[:, nb * GC + hh * CH: nb * GC + (hh + 1) * CH],
                        k_sb[:, nb * GC + hh * CH: nb * GC + (hh + 1) * CH],
                        start=(nb == 0), stop=(nb == NB - 1))

                # z = S * (temp * rinvk[d]);  E = exp(z * rinvq[c]); sum over d
                # (only one tensor_tensor input may come from PSUM -> R via SBUF)
                r_sb = small.tile([CH, CH], F32, tag="rsb")
                nc.vector.tensor_copy(r_sb, r_ps)
                z_sb = small.tile([CH, CH], F32, tag="z")
                nc.vector.tensor_mul(z_sb, s_ps, r_sb)
                e_sb = small.tile([CH, CH], BF, tag="e")
                sume = small.tile([CH, 1], F32, tag="se")
                nc.scalar.activation(e_sb, z_sb, mybir.ActivationFunctionType.Exp,
                                     scale=rq_col, accum_out=sume)
                rden = small.tile([CH, 1], F32, tag="rd")
                nc.vector.reciprocal(rden, sume)

                # normalize rows once (ACT Copy with per-partition scale), then
                # transpose attn on PE
                attn_s = small.tile([CH, CH], BF, tag="at")
                nc.scalar.activation(attn_s, e_sb,
                                     mybir.ActivationFunctionType.Copy,
                                     scale=rden)
                et_ps = tinyps.tile([CH, CH], BF, tag="tp")
                nc.tensor.transpose(et_ps, attn_s, ident96)
                et_sb = small.tile([CH, CH], BF, tag="et")
                nc.vector.tensor_copy(et_sb, et_ps)

                # out_h = attn^T^T @ vT -> ot (channel-major).  SBUF engine APs
                # must start at a 32-partition boundary (<=32 rows when starting
                # off-0), so evict in 32-row pieces.
                s0 = h * CH
                for n5 in range(N5):
                    op_ = hps.tile([CH, 512], F32, tag="hp")
                    nc.tensor.matmul(op_, et_sb, vt_sb[:, n5 * 512:(n5 + 1) * 512],
                                     start=True, stop=True)
                    for pc in range(3):
                        k0p, o = divmod(s0 + pc * 32, P)
                        nc.vector.tensor_copy(
                            ot[o:o + 32, k0p * N + n5 * 512: k0p * N + (n5 + 1) * 512],
                            op_[pc * 32:(pc + 1) * 32])

        gctx.close()

        # ---- Phase PROJ: y = OT^T @ Wproj + bproj ----
        with tc.tile_pool(name="wp", bufs=1) as wp_pool, \
             tc.tile_pool(name="wpstage", bufs=2) as wpstage, \
             tc.tile_pool(name="yout", bufs=3) as yout, \
             tc.tile_pool(name="yps", bufs=2, space="PSUM") as yps:
            wp = wp_pool.tile([P, KC * C], BF)
            for kc in range(KC):
                st = wpstage.tile([P, C], F32, tag="wpst")
                nc.sync.dma_start(out=st, in_=wproj_d[kc * P:(kc + 1) * P, :])
                nc.vector.tensor_copy(wp[:, kc * C:(kc + 1) * C], st)

            for nb in range(NB):
                y1 = yps.tile([P, 512], F32, tag="y1")
                y2 = yps.tile([P, 256], F32, tag="y2")
                for kc in range(KC):
                    lhsT = ot[:, kc * N + nb * P: kc * N + (nb + 1) * P]
                    nc.tensor.matmul(y1, lhsT, wp[:, kc * C: kc * C + 512],
                                     start=(kc == 0), stop=False)
                    nc.tensor.matmul(y2, lhsT, wp[:, kc * C + 512: (kc + 1) * C],
                                     start=(kc == 0), stop=False)
                nc.tensor.matmul(y1, ones_row, bproj_bf[0:1, 0:512],
                                 start=False, stop=True)
                nc.tensor.matmul(y2, ones_row, bproj_bf[0:1, 512:C],
                                 start=False, stop=True)
                ysb = yout.tile([P, C], F32, tag="y")
                nc.vector.tensor_copy(ysb[:, 0:512], y1)
                nc.vector.tensor_copy(ysb[:, 512:C], y2)
                nc.sync.dma_start(out=y_d[nb * P:(nb + 1) * P, :], in_=ysb)

    # Split multi-wait sync conditions into EventSemaphore instructions —
    # walrus' ACT/DVE instruction structs encode at most one wait.
    bass_rust.generate_event_semaphores(nc)
    return nc


def _in_maps(x, Wqkv, temperature, Wproj, bproj):
    wqkv = np.ascontiguousarray(Wqkv, dtype=np.float32)
    temp = np.ascontiguousarray(temperature, dtype=np.float32).reshape(H)
    wproj = np.ascontiguousarray(Wproj, dtype=np.float32)
    bp = np.ascontiguousarray(bproj, dtype=np.float32)
    return [
        {"x": np.ascontiguousarray(x[b], dtype=np.float32), "Wqkv": wqkv,
         "temperature": temp, "Wproj": wproj, "bproj": bp}
        for b in range(x.shape[0])
    ]


def run(x, Wqkv, temperature, Wproj, bproj, trace=False):
    nc = build_nc()
    in_maps = _in_maps(x, Wqkv, temperature, Wproj, bproj)
    res = run_bass_kernel_spmd(nc, in_maps, core_ids=list(range(len(in_maps))),
                               trace=trace)
    out = np.stack([res.results[b]["y"] for b in range(len(in_maps))], axis=0)
    return out.astype(np.float32), res


def kernel(x, Wqkv, temperature, Wproj, bproj):
    out, _ = run(x, Wqkv, temperature, Wproj, bproj, trace=False)
    return out


# revision 19
# speedup vs baseline: 1.1903x; 1.1903x over previous
"""Channel-attention (XCA) block on 8 trn2 NeuronCores, data-parallel over batch.

Per core: x (4096, 768) -> qkv -> per-head channel attention (96x96 scores over
l2-normalized q,k transposed to (Ch, N)) -> proj.  All big matmuls run in bf16
with fp32 PSUM accumulation; norms/softmax in fp32.

v2: head-major attention-output layout (one eviction per tile), S-before-vT
software pipeline per head (softmax latency hidden under vT matmuls, PE stays
HAM-warm), combined q|k PSUM eviction, squares on the scalar engine.
"""

import numpy as np
from contextlib import ExitStack

import bass_rust
import concourse.bass as bass
import concourse.tile as tile
from concourse import mybir
from concourse.masks import make_identity
from concourse.bass_utils import run_bass_kernel_spmd

F32 = mybir.dt.float32
BF = mybir.dt.bfloat16
AF = mybir.ActivationFunctionType

P = 128          # partitions
N = 4096         # tokens per core (batch element)
C = 768          # channels
H = 8            # heads
CH = 96          # channels per head
KC = C // P      # 6 contraction chunks of 128
NB = N // P      # 32 token blocks of 128
N5 = N // 512    # 8 token blocks of 512
G = 4            # head groups
HPG = H // G     # 2 heads per group
GC = HPG * CH    # 192 qkv columns per group
EPS = 1e-12


def build_nc():
    nc = bass.Bass()

    x_d = nc.dram_tensor("x", [N, C], F32, kind="ExternalInput")
    wqkv_d = nc.dram_tensor("Wqkv", [C, 3 * C], F32, kind="ExternalInput")
    temp_d = nc.dram_tensor("temperature", [H], F32, kind="ExternalInput")
    wproj_d = nc.dram_tensor("Wproj", [C, C], F32, kind="ExternalInput")
    bproj_d = nc.dram_tensor("bproj", [C], F32, kind="ExternalInput")
    y_d = nc.dram_tensor("y", [N, C], F32, kind="ExternalOutput")

    with ExitStack() as ctx:
        tc = ctx.enter_context(tile.TileContext(nc))
        persist = ctx.enter_context(tc.tile_pool(name="persist", bufs=1))

        # persistent SBUF: xT[c%128, (c//128)*N + n] = x[n, c]  (bf16)
        xT = persist.tile([P, KC * N], BF)
        # Wqkv bf16: wq[c%128, (c//128)*2304 + j] = Wqkv[c, j]
        wq = persist.tile([P, KC * 3 * C], BF)
        # attention output, head-major: ot[c, h*N + n] = O[n, h*CH + c]
        ot = persist.tile([CH, H * N], BF)

        ident128 = persist.tile([P, P], F32)
        make_identity(nc, ident128)
        ident96 = persist.tile([CH, CH], BF)
        make_identity(nc, ident96)
        ones_col = persist.tile([P, 1], BF)      # norm-matmul lhsT (K=128, M=1)
        nc.vector.memset(ones_col, 1.0)
        ones_row = persist.tile([1, P], BF)      # bias-matmul lhsT (K=1, M=128)
        nc.vector.memset(ones_row, 1.0)
        one1 = persist.tile([1, 1], F32)         # row->col matmul rhs
        nc.vector.memset(one1, 1.0)
        ones96 = persist.tile([1, CH], F32)
        nc.vector.memset(ones96, 1.0)

        temp_sb = persist.tile([1, H], F32)
        nc.sync.dma_start(out=temp_sb, in_=temp_d.rearrange("(a h) -> a h", a=1))
        bstage = persist.tile([1, C], F32)
        nc.sync.dma_start(out=bstage, in_=bproj_d.rearrange("(a c) -> a c", a=1))
        bproj_bf = persist.tile([1, C], BF)
        nc.vector.tensor_copy(bproj_bf, bstage)

        # ---- Phase W: load Wqkv, cast to bf16 ----
        with tc.tile_pool(name="wstage", bufs=2) as wstage:
            for kc in range(KC):
                st = wstage.tile([P, 3 * C], F32, tag="wst")
                nc.sync.dma_start(out=st, in_=wqkv_d[kc * P:(kc + 1) * P, :])
                nc.vector.tensor_copy(wq[:, kc * 3 * C:(kc + 1) * 3 * C], st)

        # ---- Phase T: x -> xT (PE transpose, downcast on evict) ----
        # 4+2 transposes share two PSUM tiles (one bank each) so the nb's
        # eviction is 2 copies instead of 6.
        with tc.tile_pool(name="xstage", bufs=3) as xstage, \
             tc.tile_pool(name="tps", bufs=2, space="PSUM") as tps:
            for nb in range(NB):
                xt_ = xstage.tile([P, C], F32, tag="x")
                nc.sync.dma_start(out=xt_, in_=x_d[nb * P:(nb + 1) * P, :])
                t1 = tps.tile([P, 512], F32, tag="t1")
                t2 = tps.tile([P, 256], F32, tag="t2")
                for kc in range(KC):
                    dst = t1[:, (kc % 4) * P:(kc % 4 + 1) * P] if kc < 4 \
                        else t2[:, (kc - 4) * P:(kc - 3) * P]
                    nc.tensor.matmul(dst, xt_[:, kc * P:(kc + 1) * P], ident128,
                                     is_transpose=True,
                                     start=(kc in (0, 4)), stop=(kc in (3, 5)))
                for kc in range(KC):
                    nc.vector.tensor_copy(
                        xT[:, kc * N + nb * P: kc * N + (nb + 1) * P],
                        (t1[:, (kc % 4) * P:(kc % 4 + 1) * P] if kc < 4
                         else t2[:, (kc - 4) * P:(kc - 3) * P]))

        # ---- head-group loop ----
        gctx = ctx.enter_context(ExitStack())
        qk_pool = gctx.enter_context(tc.tile_pool(name="qk", bufs=1))
        qkps = gctx.enter_context(tc.tile_pool(name="qkps", bufs=2, space="PSUM"))
        nrmps = gctx.enter_context(tc.tile_pool(name="nrmps", bufs=1, space="PSUM"))
        sqpool = gctx.enter_context(tc.tile_pool(name="sq", bufs=3))
        small = gctx.enter_context(tc.tile_pool(name="small", bufs=2))
        vt_pool = gctx.enter_context(tc.tile_pool(name="vt", bufs=2))
        hps = gctx.enter_context(tc.tile_pool(name="hps", bufs=2, space="PSUM"))
        sps = gctx.enter_context(tc.tile_pool(name="sps", bufs=2, space="PSUM"))
        tinyps = gctx.enter_context(tc.tile_pool(name="tinyps", bufs=1, space="PSUM"))

        for g in range(G):
            # qk_sb: (128 n, nb*(2*GC) + [q cols | k cols]) bf16
            qk_sb = qk_pool.tile([P, NB * 2 * GC], BF, tag="qk")
            nqk = nrmps.tile([1, 2 * GC], F32, tag="nqk")  # [q sumsq | k sumsq]

            for nb in range(NB):
                qkp = qkps.tile([P, 2 * GC], F32, tag="qkp")
                qp = qkp[:, 0:GC]
                kp = qkp[:, GC:2 * GC]
                # qp|kp live in one PSUM bank: one accumulation group (start
                # pending-zeroes the whole bank, k region accumulates onto 0)
                for kc in range(KC):
                    lhsT = xT[:, kc * N + nb * P: kc * N + (nb + 1) * P]
                    nc.tensor.matmul(
                        qp, lhsT, wq[:, kc * 3 * C + g * GC: kc * 3 * C + (g + 1) * GC],
                        start=(kc == 0), stop=False)
                    nc.tensor.matmul(
                        kp, lhsT,
                        wq[:, kc * 3 * C + C + g * GC: kc * 3 * C + C + (g + 1) * GC],
                        start=False, stop=(kc == KC - 1))
                qks = qk_sb[:, nb * 2 * GC:(nb + 1) * 2 * GC]
                nc.vector.tensor_copy(qks, qkp)
                sq = sqpool.tile([P, 2 * GC], BF, tag="sq")
                nc.scalar.activation(sq, qks, AF.Square)
                nc.tensor.matmul(nqk[0:1, 0:GC], ones_col, sq[:, 0:GC],
                                 start=(nb == 0), stop=False)
                nc.tensor.matmul(nqk[0:1, GC:2 * GC], ones_col, sq[:, GC:2 * GC],
                                 start=False, stop=(nb == NB - 1))

            # scores first (PE busy while the norm chain below finishes)
            s_list = []
            for hh in range(HPG):
                s_ps = sps.tile([CH, CH], F32, tag="s")
                for nb in range(NB):
                    base = nb * 2 * GC
                    nc.tensor.matmul(
                        s_ps,
                        qk_sb[:, base + hh * CH: base + (hh + 1) * CH],
                        qk_sb[:, base + GC + hh * CH: base + GC + (hh + 1) * CH],
                        start=(nb == 0), stop=(nb == NB - 1))
                s_list.append(s_ps)

            # rinv row: 1 / max(sqrt(sumsq), eps), [q | k] in one (1, 384) row
            rqk = small.tile([1, 2 * GC], F32, tag="rqk")
            nc.scalar.activation(rqk, nqk, AF.Sqrt)
            nc.vector.tensor_scalar_max(rqk, rqk, EPS)
            nc.vector.reciprocal(rqk, rqk)

            # per-head norm-derived tiles (tiny PE matmuls, off the pipeline)
            rq_cols, r_sbs = [], []
            for hh in range(HPG):
                h = g * HPG + hh
                rq_ps = tinyps.tile([CH, 1], F32, tag="tp")
                nc.tensor.matmul(rq_ps, rqk[0:1, hh * CH:(hh + 1) * CH], one1,
                                 start=True, stop=True)
                rq_col = small.tile([CH, 1], F32, tag="rqc")
                nc.vector.tensor_copy(rq_col, rq_ps)
                tempb = small.tile([1, CH], F32, tag="tb")
                nc.scalar.activation(tempb, ones96, AF.Copy,
                                     scale=temp_sb[0:1, h:h + 1])
                r_ps = tinyps.tile([CH, CH], F32, tag="tp")
                nc.tensor.matmul(r_ps, tempb,
                                 rqk[0:1, GC + hh * CH: GC + (hh + 1) * CH],
                                 start=True, stop=True)
                r_sb = small.tile([CH, CH], F32, tag="rsb")
                nc.vector.tensor_copy(r_sb, r_ps)
                rq_cols.append(rq_col)
                r_sbs.append(r_sb)

            for hh in range(HPG):
                h = g * HPG + hh

                # softmax chain on DVE/ACT — overlaps the vT matmuls below
                z_sb = small.tile([CH, CH], F32, tag="z")
                nc.vector.tensor_mul(z_sb, s_list[hh], r_sbs[hh])
                e_sb = small.tile([CH, CH], BF, tag="e")
                sume = small.tile([CH, 1], F32, tag="se")
                nc.scalar.activation(e_sb, z_sb, AF.Exp,
                                     scale=rq_cols[hh], accum_out=sume)
                rden = small.tile([CH, 1], F32, tag="rd")
                nc.vector.reciprocal(rden, sume)
                attn_s = small.tile([CH, CH], BF, tag="at")
                nc.scalar.activation(attn_s, e_sb, AF.Copy, scale=rden)

                # vT for this head: (96 d, 4096 n) bf16, from Wqkv v-cols and xT
                vt_sb = vt_pool.tile([CH, N], BF, tag="vt")
                for n5 in range(N5):
                    vp = hps.tile([CH, 512], F32, tag="hp")
                    for kc in range(KC):
                        nc.tensor.matmul(
                            vp,
                            wq[:, kc * 3 * C + 2 * C + h * CH: kc * 3 * C + 2 * C + (h + 1) * CH],
                            xT[:, kc * N + n5 * 512: kc * N + (n5 + 1) * 512],
                            start=(kc == 0), stop=(kc == KC - 1))
                    nc.vector.tensor_copy(vt_sb[:, n5 * 512:(n5 + 1) * 512], vp)

                # attn^T via PE transpose (ready well before vT finishes)
                et_ps = tinyps.tile([CH, CH], BF, tag="tp")
                nc.tensor.transpose(et_ps, attn_s, ident96)
                et_sb = small.tile([CH, CH], BF, tag="et")
                nc.vector.tensor_copy(et_sb, et_ps)

                # out_h = attn @ vT -> ot[:, h*N + n] (single copy per tile)
                for n5 in range(N5):
                    op_ = hps.tile([CH, 512], F32, tag="hp")
                    nc.tensor.matmul(op_, et_sb, vt_sb[:, n5 * 512:(n5 + 1) * 512],
                                     start=True, stop=True)
                    nc.vector.tensor_copy(
                        ot[:, h * N + n5 * 512: h * N + (n5 + 1) * 512], op_)

        gctx.close()

        # ---- Phase PROJ: y = OT^T @ Wproj + bproj (8 K=96 chunks, head-major)
        with tc.tile_pool(name="wp", bufs=1) as wp_pool, \
             tc.tile_pool(name="wpstage", bufs=2) as wpstage, \
             tc.tile_pool(name="yout", bufs=3) as yout, \
             tc.tile_pool(name="yps", bufs=2, space="PSUM") as yps:
            wp = wp_pool.tile([CH, H * C], BF)   # wp[c, h*C + j] = Wproj[h*CH+c, j]
            for h in range(H):
                st = wpstage.tile([CH, C], F32, tag="wpst")
                nc.sync.dma_start(out=st, in_=wproj_d[h * CH:(h + 1) * CH, :])
                nc.vector.tensor_copy(wp[:, h * C:(h + 1) * C], st)

            for nb in range(NB):
                y1 = yps.tile([P, 512], F32, tag="y1")
                y2 = yps.tile([P, 256], F32, tag="y2")
                for h in range(H):
                    lhsT = ot[:, h * N + nb * P: h * N + (nb + 1) * P]
                    nc.tensor.matmul(y1, lhsT, wp[:, h * C: h * C + 512],
                                     start=(h == 0), stop=False)
                    nc.tensor.matmul(y2, lhsT, wp[:, h * C + 512: (h + 1) * C],
                                     start=(h == 0), stop=False)
                nc.tensor.matmul(y1, ones_row, bproj_bf[0:1, 0:512],
                                 start=False, stop=True)
                nc.tensor.matmul(y2, ones_row, bproj_bf[0:1, 512:C],
                                 start=False, stop=True)
                ysb = yout.tile([P, C], F32, tag="y")
                nc.vector.tensor_copy(ysb[:, 0:512], y1)
                nc.vector.tensor_copy(ysb[:, 512:C], y2)
                nc.sync.dma_start(out=y_d[nb * P:(nb + 1) * P, :], in_=ysb)

    # Split multi-wait sync conditions into EventSemaphore instructions —
    # walrus' ACT/DVE instruction structs encode at most one wait.
    bass_rust.generate_event_semaphores(nc)
    return nc


def _in_maps(x, Wqkv, temperature, Wproj, bproj):
    wqkv = np.ascontiguousarray(Wqkv, dtype=np.float32)
    temp = np.ascontiguousarray(temperature, dtype=np.float32).reshape(H)
    wproj = np.ascontiguousarray(Wproj, dtype=np.float32)
    bp = np.ascontiguousarray(bproj, dtype=np.float32)
    return [
        {"x": np.ascontiguousarray(x[b], dtype=np.float32), "Wqkv": wqkv,
         "temperature": temp, "Wproj": wproj, "bproj": bp}
        for b in range(x.shape[0])
    ]


def run(x, Wqkv, temperature, Wproj, bproj, trace=False):
    nc = build_nc()
    in_maps = _in_maps(x, Wqkv, temperature, Wproj, bproj)
    res = run_bass_kernel_spmd(nc, in_maps, core_ids=list(range(len(in_maps))),
                               trace=trace)
    out = np.stack([res.results[b]["y"] for b in range(len(in_maps))], axis=0)
    return out.astype(np.float32), res


def kernel(x, Wqkv, temperature, Wproj, bproj):
    out, _ = run(x, Wqkv, temperature, Wproj, bproj, trace=False)
    return out


# revision 20
# speedup vs baseline: 1.2612x; 1.0595x over previous
"""Channel-attention (XCA) block on 8 trn2 NeuronCores, data-parallel over batch.

Per core: x (4096, 768) -> qkv -> per-head channel attention (96x96 scores over
l2-normalized q,k transposed to (Ch, N)) -> proj.  All big matmuls run in bf16
with fp32 PSUM accumulation; norms/softmax in fp32.

v3: group-0 qk generation fused into the x-transpose loop (PE dense from the
start, HAM stays warm), head-major attention-output layout, S-before-vT
software pipeline per head (softmax latency hidden under vT matmuls), single
eviction copies via 3D tiles.
"""

import numpy as np
from contextlib import ExitStack

import bass_rust
import concourse.bass as bass
import concourse.tile as tile
from concourse import mybir
from concourse.masks import make_identity
from concourse.bass_utils import run_bass_kernel_spmd

F32 = mybir.dt.float32
BF = mybir.dt.bfloat16
AF = mybir.ActivationFunctionType

P = 128          # partitions
N = 4096         # tokens per core (batch element)
C = 768          # channels
H = 8            # heads
CH = 96          # channels per head
KC = C // P      # 6 contraction chunks of 128
NB = N // P      # 32 token blocks of 128
N5 = N // 512    # 8 token blocks of 512
G = 4            # head groups
HPG = H // G     # 2 heads per group
GC = HPG * CH    # 192 qkv columns per group
EPS = 1e-12


def build_nc():
    nc = bass.Bass()

    x_d = nc.dram_tensor("x", [N, C], F32, kind="ExternalInput")
    wqkv_d = nc.dram_tensor("Wqkv", [C, 3 * C], F32, kind="ExternalInput")
    temp_d = nc.dram_tensor("temperature", [H], F32, kind="ExternalInput")
    wproj_d = nc.dram_tensor("Wproj", [C, C], F32, kind="ExternalInput")
    bproj_d = nc.dram_tensor("bproj", [C], F32, kind="ExternalInput")
    y_d = nc.dram_tensor("y", [N, C], F32, kind="ExternalOutput")

    with ExitStack() as ctx:
        tc = ctx.enter_context(tile.TileContext(nc))
        persist = ctx.enter_context(tc.tile_pool(name="persist", bufs=1))

        # persistent SBUF: xT[c%128, c//128, n] = x[n, c]  (bf16)
        xT = persist.tile([P, KC, N], BF)
        # Wqkv bf16: wq[c%128, c//128, j] = Wqkv[c, j]
        wq = persist.tile([P, KC, 3 * C], BF)
        # attention output, head-major: ot[c, h, n] = O[n, h*CH + c]
        ot = persist.tile([CH, H, N], BF)

        ident128 = persist.tile([P, P], F32)
        make_identity(nc, ident128)
        ident96 = persist.tile([CH, CH], BF)
        make_identity(nc, ident96)
        ones_col = persist.tile([P, 1], BF)      # norm-matmul lhsT (K=128, M=1)
        nc.vector.memset(ones_col, 1.0)
        ones_row = persist.tile([1, P], BF)      # bias-matmul lhsT (K=1, M=128)
        nc.vector.memset(ones_row, 1.0)
        one1 = persist.tile([1, 1], F32)         # row->col matmul rhs
        nc.vector.memset(one1, 1.0)
        ones96 = persist.tile([1, CH], F32)
        nc.vector.memset(ones96, 1.0)

        temp_sb = persist.tile([1, H], F32)
        nc.sync.dma_start(out=temp_sb, in_=temp_d.rearrange("(a h) -> a h", a=1))
        bstage = persist.tile([1, C], F32)
        nc.sync.dma_start(out=bstage, in_=bproj_d.rearrange("(a c) -> a c", a=1))
        bproj_bf = persist.tile([1, C], BF)
        nc.vector.tensor_copy(bproj_bf, bstage)

        # ---- Phase W: load Wqkv, cast to bf16 ----
        with tc.tile_pool(name="wstage", bufs=2) as wstage:
            for kc in range(KC):
                st = wstage.tile([P, 3 * C], F32, tag="wst")
                nc.sync.dma_start(out=st, in_=wqkv_d[kc * P:(kc + 1) * P, :])
                nc.vector.tensor_copy(wq[:, kc, :], st)

        gctx = ctx.enter_context(ExitStack())
        qk_pool = gctx.enter_context(tc.tile_pool(name="qk", bufs=1))
        qkps = gctx.enter_context(tc.tile_pool(name="qkps", bufs=2, space="PSUM"))
        nrmps = gctx.enter_context(tc.tile_pool(name="nrmps", bufs=1, space="PSUM"))
        sqpool = gctx.enter_context(tc.tile_pool(name="sq", bufs=3))
        small = gctx.enter_context(tc.tile_pool(name="small", bufs=2))

        def qk_block(g, nb, qk_sb, nqk):
            """qkv q|k matmuls for one token block + eviction + norm matmul."""
            qkp = qkps.tile([P, 2 * GC], F32, tag="qkp")
            qp = qkp[:, 0:GC]
            kp = qkp[:, GC:2 * GC]
            # qp|kp live in one PSUM bank: one accumulation group (start
            # pending-zeroes the whole bank, k region accumulates onto 0)
            for kc in range(KC):
                lhsT = xT[:, kc, nb * P:(nb + 1) * P]
                nc.tensor.matmul(
                    qp, lhsT, wq[:, kc, g * GC:(g + 1) * GC],
                    start=(kc == 0), stop=False)
                nc.tensor.matmul(
                    kp, lhsT, wq[:, kc, C + g * GC: C + (g + 1) * GC],
                    start=False, stop=(kc == KC - 1))
            qks = qk_sb[:, nb, :]
            nc.vector.tensor_copy(qks, qkp)
            sq = sqpool.tile([P, 2 * GC], BF, tag="sq")
            nc.scalar.activation(sq, qks, AF.Square)
            nc.tensor.matmul(nqk, ones_col, sq,
                             start=(nb == 0), stop=(nb == NB - 1))

        def heads_phase(g, qk_sb, nqk):
            """scores, softmax (hidden under vT gen), out tiles for one group."""
            # scores first: PE stays busy while the norm chain completes
            s_list = []
            for hh in range(HPG):
                s_ps = sps.tile([CH, CH], F32, tag="s")
                for nb in range(NB):
                    nc.tensor.matmul(
                        s_ps,
                        qk_sb[:, nb, hh * CH:(hh + 1) * CH],
                        qk_sb[:, nb, GC + hh * CH: GC + (hh + 1) * CH],
                        start=(nb == 0), stop=(nb == NB - 1))
                s_list.append(s_ps)

            # rinv row: 1 / max(sqrt(sumsq), eps), [q | k] in one (1, 384) row
            rqk = small.tile([1, 2 * GC], F32, tag="rqk")
            nc.scalar.activation(rqk, nqk, AF.Sqrt)
            nc.vector.tensor_scalar_max(rqk, rqk, EPS)
            nc.vector.reciprocal(rqk, rqk)

            # per-head norm-derived tiles (tiny PE matmuls, off the pipeline)
            rq_cols, r_sbs = [], []
            for hh in range(HPG):
                h = g * HPG + hh
                rq_ps = tinyps.tile([CH, 1], F32, tag="tp")
                nc.tensor.matmul(rq_ps, rqk[0:1, hh * CH:(hh + 1) * CH], one1,
                                 start=True, stop=True)
                rq_col = small.tile([CH, 1], F32, tag="rqc")
                nc.vector.tensor_copy(rq_col, rq_ps)
                tempb = small.tile([1, CH], F32, tag="tb")
                nc.scalar.activation(tempb, ones96, AF.Copy,
                                     scale=temp_sb[0:1, h:h + 1])
                r_ps = tinyps.tile([CH, CH], F32, tag="tp")
                nc.tensor.matmul(r_ps, tempb,
                                 rqk[0:1, GC + hh * CH: GC + (hh + 1) * CH],
                                 start=True, stop=True)
                r_sb = small.tile([CH, CH], F32, tag="rsb")
                nc.vector.tensor_copy(r_sb, r_ps)
                rq_cols.append(rq_col)
                r_sbs.append(r_sb)

            for hh in range(HPG):
                h = g * HPG + hh
                # softmax chain on DVE/ACT — overlaps the vT matmuls below
                z_sb = small.tile([CH, CH], F32, tag="z")
                nc.vector.tensor_mul(z_sb, s_list[hh], r_sbs[hh])
                e_sb = small.tile([CH, CH], BF, tag="e")
                sume = small.tile([CH, 1], F32, tag="se")
                nc.scalar.activation(e_sb, z_sb, AF.Exp,
                                     scale=rq_cols[hh], accum_out=sume)
                rden = small.tile([CH, 1], F32, tag="rd")
                nc.vector.reciprocal(rden, sume)
                attn_s = small.tile([CH, CH], BF, tag="at")
                nc.scalar.activation(attn_s, e_sb, AF.Copy, scale=rden)

                # vT for this head: (96 d, 4096 n) bf16, from Wqkv v-cols and xT
                vt_sb = vt_pool.tile([CH, N], BF, tag="vt")
                for n5 in range(N5):
                    vp = hps.tile([CH, 512], F32, tag="hp")
                    for kc in range(KC):
                        nc.tensor.matmul(
                            vp, wq[:, kc, 2 * C + h * CH: 2 * C + (h + 1) * CH],
                            xT[:, kc, n5 * 512:(n5 + 1) * 512],
                            start=(kc == 0), stop=(kc == KC - 1))
                    nc.vector.tensor_copy(vt_sb[:, n5 * 512:(n5 + 1) * 512], vp)

                # attn^T via PE transpose (ready well before vT finishes)
                et_ps = tinyps.tile([CH, CH], BF, tag="tp")
                nc.tensor.transpose(et_ps, attn_s, ident96)
                et_sb = small.tile([CH, CH], BF, tag="et")
                nc.vector.tensor_copy(et_sb, et_ps)

                # out_h = attn @ vT -> ot[:, h, n] (single copy per tile)
                for n5 in range(N5):
                    op_ = hps.tile([CH, 512], F32, tag="hp")
                    nc.tensor.matmul(op_, et_sb, vt_sb[:, n5 * 512:(n5 + 1) * 512],
                                     start=True, stop=True)
                    nc.vector.tensor_copy(ot[:, h, n5 * 512:(n5 + 1) * 512], op_)

        # ---- Phase T fused with group-0 qk: x -> xT + q/k(g0), PE dense ----
        qk0 = qk_pool.tile([P, NB, 2 * GC], BF, tag="qk")
        nqk0 = nrmps.tile([1, 2 * GC], F32, tag="nqk")
        with tc.tile_pool(name="xstage", bufs=3) as xstage, \
             tc.tile_pool(name="tps", bufs=2, space="PSUM") as tps:
            for nb in range(NB):
                xt_ = xstage.tile([P, C], F32, tag="x")
                nc.sync.dma_start(out=xt_, in_=x_d[nb * P:(nb + 1) * P, :])
                tall = tps.tile([P, KC, P], F32, tag="t")  # 2 banks: kc 0-3 | 4-5
                for kc in range(KC):
                    nc.tensor.matmul(tall[:, kc, :], xt_[:, kc * P:(kc + 1) * P],
                                     ident128, is_transpose=True,
                                     start=(kc in (0, 4)), stop=(kc in (3, 5)))
                nc.vector.tensor_copy(xT[:, :, nb * P:(nb + 1) * P], tall)
                qk_block(0, nb, qk0, nqk0)

        # heads pools open after the transpose PSUM pool closes (bank budget)
        vt_pool = gctx.enter_context(tc.tile_pool(name="vt", bufs=2))
        hps = gctx.enter_context(tc.tile_pool(name="hps", bufs=2, space="PSUM"))
        sps = gctx.enter_context(tc.tile_pool(name="sps", bufs=2, space="PSUM"))
        tinyps = gctx.enter_context(tc.tile_pool(name="tinyps", bufs=1, space="PSUM"))

        heads_phase(0, qk0, nqk0)
        for g in range(1, G):
            qk_sb = qk_pool.tile([P, NB, 2 * GC], BF, tag="qk")
            nqk = nrmps.tile([1, 2 * GC], F32, tag="nqk")
            for nb in range(NB):
                qk_block(g, nb, qk_sb, nqk)
            heads_phase(g, qk_sb, nqk)

        gctx.close()

        # ---- Phase PROJ: y = OT^T @ Wproj + bproj (8 K=96 chunks, head-major)
        with tc.tile_pool(name="wp", bufs=1) as wp_pool, \
             tc.tile_pool(name="wpstage", bufs=2) as wpstage, \
             tc.tile_pool(name="yout", bufs=3) as yout, \
             tc.tile_pool(name="yps", bufs=2, space="PSUM") as yps:
            wp = wp_pool.tile([CH, H, C], BF)   # wp[c, h, j] = Wproj[h*CH+c, j]
            for h in range(H):
                st = wpstage.tile([CH, C], F32, tag="wpst")
                nc.sync.dma_start(out=st, in_=wproj_d[h * CH:(h + 1) * CH, :])
                nc.vector.tensor_copy(wp[:, h, :], st)

            for nb in range(NB):
                y1 = yps.tile([P, 512], F32, tag="y1")
                y2 = yps.tile([P, 256], F32, tag="y2")
                for h in range(H):
                    lhsT = ot[:, h, nb * P:(nb + 1) * P]
                    nc.tensor.matmul(y1, lhsT, wp[:, h, 0:512],
                                     start=(h == 0), stop=False)
                    nc.tensor.matmul(y2, lhsT, wp[:, h, 512:C],
                                     start=(h == 0), stop=False)
                nc.tensor.matmul(y1, ones_row, bproj_bf[0:1, 0:512],
                                 start=False, stop=True)
                nc.tensor.matmul(y2, ones_row, bproj_bf[0:1, 512:C],
                                 start=False, stop=True)
                ysb = yout.tile([P, C], F32, tag="y")
                nc.vector.tensor_copy(ysb[:, 0:512], y1)
                nc.vector.tensor_copy(ysb[:, 512:C], y2)
                nc.sync.dma_start(out=y_d[nb * P:(nb + 1) * P, :], in_=ysb)

    # Split multi-wait sync conditions into EventSemaphore instructions —
    # walrus' ACT/DVE instruction structs encode at most one wait.
    bass_rust.generate_event_semaphores(nc)
    return nc


def _in_maps(x, Wqkv, temperature, Wproj, bproj):
    wqkv = np.ascontiguousarray(Wqkv, dtype=np.float32)
    temp = np.ascontiguousarray(temperature, dtype=np.float32).reshape(H)
    wproj = np.ascontiguousarray(Wproj, dtype=np.float32)
    bp = np.ascontiguousarray(bproj, dtype=np.float32)
    return [
        {"x": np.ascontiguousarray(x[b], dtype=np.float32), "Wqkv": wqkv,
         "temperature": temp, "Wproj": wproj, "bproj": bp}
        for b in range(x.shape[0])
    ]


def run(x, Wqkv, temperature, Wproj, bproj, trace=False):
    nc = build_nc()
    in_maps = _in_maps(x, Wqkv, temperature, Wproj, bproj)
    res = run_bass_kernel_spmd(nc, in_maps, core_ids=list(range(len(in_maps))),
                               trace=trace)
    out = np.stack([res.results[b]["y"] for b in range(len(in_maps))], axis=0)
    return out.astype(np.float32), res


def kernel(x, Wqkv, temperature, Wproj, bproj):
    out, _ = run(x, Wqkv, temperature, Wproj, bproj, trace=False)
    return out


# revision 22
# speedup vs baseline: 1.2813x; 1.0159x over previous
"""Channel-attention (XCA) block on 8 trn2 NeuronCores, data-parallel over batch.

Per core: x (4096, 768) -> qkv -> per-head channel attention (96x96 scores over
l2-normalized q,k transposed to (Ch, N)) -> proj.  All big matmuls run in bf16
with fp32 PSUM accumulation; norms/softmax in fp32.

v3: group-0 qk generation fused into the x-transpose loop (PE dense from the
start, HAM stays warm), head-major attention-output layout, S-before-vT
software pipeline per head (softmax latency hidden under vT matmuls), single
eviction copies via 3D tiles.
"""

import numpy as np
from contextlib import ExitStack

import bass_rust
import concourse.bass as bass
import concourse.tile as tile
from concourse import mybir
from concourse.masks import make_identity
from concourse.bass_utils import run_bass_kernel_spmd

F32 = mybir.dt.float32
BF = mybir.dt.bfloat16
AF = mybir.ActivationFunctionType

P = 128          # partitions
N = 4096         # tokens per core (batch element)
C = 768          # channels
H = 8            # heads
CH = 96          # channels per head
KC = C // P      # 6 contraction chunks of 128
NB = N // P      # 32 token blocks of 128
N5 = N // 512    # 8 token blocks of 512
G = 4            # head groups
HPG = H // G     # 2 heads per group
GC = HPG * CH    # 192 qkv columns per group
EPS = 1e-12


def build_nc():
    nc = bass.Bass()

    x_d = nc.dram_tensor("x", [N, C], F32, kind="ExternalInput")
    wqkv_d = nc.dram_tensor("Wqkv", [C, 3 * C], F32, kind="ExternalInput")
    temp_d = nc.dram_tensor("temperature", [H], F32, kind="ExternalInput")
    wproj_d = nc.dram_tensor("Wproj", [C, C], F32, kind="ExternalInput")
    bproj_d = nc.dram_tensor("bproj", [C], F32, kind="ExternalInput")
    y_d = nc.dram_tensor("y", [N, C], F32, kind="ExternalOutput")

    with ExitStack() as ctx:
        tc = ctx.enter_context(tile.TileContext(nc))
        persist = ctx.enter_context(tc.tile_pool(name="persist", bufs=1))

        # persistent SBUF: xT[c%128, c//128, n] = x[n, c]  (bf16)
        xT = persist.tile([P, KC, N], BF)
        # Wqkv bf16: wq[c%128, c//128, j] = Wqkv[c, j]
        wq = persist.tile([P, KC, 3 * C], BF)
        # attention output, head-major: ot[c, h, n] = O[n, h*CH + c]
        ot = persist.tile([CH, H, N], BF)

        ident128 = persist.tile([P, P], F32)
        make_identity(nc, ident128)
        ident96 = persist.tile([CH, CH], BF)
        make_identity(nc, ident96)
        ones_col = persist.tile([P, 1], BF)      # norm-matmul lhsT (K=128, M=1)
        nc.vector.memset(ones_col, 1.0)
        ones_row = persist.tile([1, P], BF)      # bias-matmul lhsT (K=1, M=128)
        nc.vector.memset(ones_row, 1.0)
        one1 = persist.tile([1, 1], F32)         # row->col matmul rhs
        nc.vector.memset(one1, 1.0)
        ones96 = persist.tile([1, CH], F32)
        nc.vector.memset(ones96, 1.0)

        temp_sb = persist.tile([1, H], F32)
        nc.sync.dma_start(out=temp_sb, in_=temp_d.rearrange("(a h) -> a h", a=1))
        bstage = persist.tile([1, C], F32)
        nc.sync.dma_start(out=bstage, in_=bproj_d.rearrange("(a c) -> a c", a=1))
        bproj_bf = persist.tile([1, C], BF)
        nc.vector.tensor_copy(bproj_bf, bstage)

        gctx = ctx.enter_context(ExitStack())
        qk_pool = gctx.enter_context(tc.tile_pool(name="qk", bufs=1))
        qkps = gctx.enter_context(tc.tile_pool(name="qkps", bufs=2, space="PSUM"))
        nrmps = gctx.enter_context(tc.tile_pool(name="nrmps", bufs=1, space="PSUM"))
        sqpool = gctx.enter_context(tc.tile_pool(name="sq", bufs=3))
        small = gctx.enter_context(tc.tile_pool(name="small", bufs=2))

        def qk_block(g, nb, qk_sb, nqk):
            """qkv q|k matmuls for one token block + eviction + norm matmul."""
            qkp = qkps.tile([P, 2 * GC], F32, tag="qkp")
            qp = qkp[:, 0:GC]
            kp = qkp[:, GC:2 * GC]
            # qp|kp live in one PSUM bank: one accumulation group (start
            # pending-zeroes the whole bank, k region accumulates onto 0)
            for kc in range(KC):
                lhsT = xT[:, kc, nb * P:(nb + 1) * P]
                nc.tensor.matmul(
                    qp, lhsT, wq[:, kc, g * GC:(g + 1) * GC],
                    start=(kc == 0), stop=False)
                nc.tensor.matmul(
                    kp, lhsT, wq[:, kc, C + g * GC: C + (g + 1) * GC],
                    start=False, stop=(kc == KC - 1))
            qks = qk_sb[:, nb, :]
            nc.vector.tensor_copy(qks, qkp)
            sq = sqpool.tile([P, 2 * GC], BF, tag="sq")
            nc.scalar.activation(sq, qks, AF.Square)
            nc.tensor.matmul(nqk, ones_col, sq,
                             start=(nb == 0), stop=(nb == NB - 1))

        def heads_phase(g, qk_sb, nqk):
            """scores, softmax (hidden under vT gen), out tiles for one group."""
            # scores first: PE stays busy while the norm chain completes
            s_list = []
            for hh in range(HPG):
                s_ps = sps.tile([CH, CH], F32, tag="s")
                for nb in range(NB):
                    nc.tensor.matmul(
                        s_ps,
                        qk_sb[:, nb, hh * CH:(hh + 1) * CH],
                        qk_sb[:, nb, GC + hh * CH: GC + (hh + 1) * CH],
                        start=(nb == 0), stop=(nb == NB - 1))
                s_list.append(s_ps)

            # rinv row: 1 / max(sqrt(sumsq), eps), [q | k] in one (1, 384) row
            rqk = small.tile([1, 2 * GC], F32, tag="rqk")
            nc.scalar.activation(rqk, nqk, AF.Sqrt)
            nc.vector.tensor_scalar_max(rqk, rqk, EPS)
            nc.vector.reciprocal(rqk, rqk)

            # per-head norm-derived tiles (tiny PE matmuls, off the pipeline)
            rq_cols, r_sbs = [], []
            for hh in range(HPG):
                h = g * HPG + hh
                rq_ps = tinyps.tile([CH, 1], F32, tag="tp")
                nc.tensor.matmul(rq_ps, rqk[0:1, hh * CH:(hh + 1) * CH], one1,
                                 start=True, stop=True)
                rq_col = small.tile([CH, 1], F32, tag="rqc")
                nc.vector.tensor_copy(rq_col, rq_ps)
                tempb = small.tile([1, CH], F32, tag="tb")
                nc.scalar.activation(tempb, ones96, AF.Copy,
                                     scale=temp_sb[0:1, h:h + 1])
                r_ps = tinyps.tile([CH, CH], F32, tag="tp")
                nc.tensor.matmul(r_ps, tempb,
                                 rqk[0:1, GC + hh * CH: GC + (hh + 1) * CH],
                                 start=True, stop=True)
                r_sb = small.tile([CH, CH], F32, tag="rsb")
                nc.vector.tensor_copy(r_sb, r_ps)
                rq_cols.append(rq_col)
                r_sbs.append(r_sb)

            for hh in range(HPG):
                h = g * HPG + hh
                # softmax chain on DVE/ACT — overlaps the vT matmuls below
                z_sb = small.tile([CH, CH], F32, tag="z")
                nc.vector.tensor_mul(z_sb, s_list[hh], r_sbs[hh])
                e_sb = small.tile([CH, CH], BF, tag="e")
                sume = small.tile([CH, 1], F32, tag="se")
                nc.scalar.activation(e_sb, z_sb, AF.Exp,
                                     scale=rq_cols[hh], accum_out=sume)
                rden = small.tile([CH, 1], F32, tag="rd")
                nc.vector.reciprocal(rden, sume)
                attn_s = small.tile([CH, CH], BF, tag="at")
                nc.scalar.activation(attn_s, e_sb, AF.Copy, scale=rden)

                # vT for this head: (96 d, 4096 n) bf16, from Wqkv v-cols and xT
                vt_sb = vt_pool.tile([CH, N], BF, tag="vt")
                for n5 in range(N5):
                    vp = hps.tile([CH, 512], F32, tag="hp")
                    for kc in range(KC):
                        nc.tensor.matmul(
                            vp, wq[:, kc, 2 * C + h * CH: 2 * C + (h + 1) * CH],
                            xT[:, kc, n5 * 512:(n5 + 1) * 512],
                            start=(kc == 0), stop=(kc == KC - 1))
                    nc.vector.tensor_copy(vt_sb[:, n5 * 512:(n5 + 1) * 512], vp)

                # attn^T via PE transpose (ready well before vT finishes)
                et_ps = tinyps.tile([CH, CH], BF, tag="tp")
                nc.tensor.transpose(et_ps, attn_s, ident96)
                et_sb = small.tile([CH, CH], BF, tag="et")
                nc.vector.tensor_copy(et_sb, et_ps)

                # out_h = attn @ vT -> ot[:, h, n] (single copy per tile)
                for n5 in range(N5):
                    op_ = hps.tile([CH, 512], F32, tag="hp")
                    nc.tensor.matmul(op_, et_sb, vt_sb[:, n5 * 512:(n5 + 1) * 512],
                                     start=True, stop=True)
                    nc.vector.tensor_copy(ot[:, h, n5 * 512:(n5 + 1) * 512], op_)

        # ---- Phase T fused with group-0 qk: x -> xT + q/k(g0), PE dense ----
        # x loads start before the 7MB Wqkv load so the transposes (and HAM
        # warmup) begin immediately; qk blocks trail by 3 iterations.
        LAG = 3
        qk0 = qk_pool.tile([P, NB, 2 * GC], BF, tag="qk")
        nqk0 = nrmps.tile([1, 2 * GC], F32, tag="nqk")
        with tc.tile_pool(name="xstage", bufs=4) as xstage, \
             tc.tile_pool(name="wstage", bufs=2) as wstage, \
             tc.tile_pool(name="tps", bufs=2, space="PSUM") as tps:
            for nb in range(NB + LAG):
                if nb < NB:
                    xt_ = xstage.tile([P, C], F32, tag="x")
                    nc.sync.dma_start(out=xt_, in_=x_d[nb * P:(nb + 1) * P, :])
                    tall = tps.tile([P, KC, P], F32, tag="t")  # banks: kc 0-3|4-5
                    for kc in range(KC):
                        nc.tensor.matmul(tall[:, kc, :], xt_[:, kc * P:(kc + 1) * P],
                                         ident128, is_transpose=True,
                                         start=(kc in (0, 4)), stop=(kc in (3, 5)))
                    nc.vector.tensor_copy(xT[:, :, nb * P:(nb + 1) * P], tall)
                if nb == LAG - 1:
                    for kc in range(KC):
                        st = wstage.tile([P, 3 * C], F32, tag="wst")
                        nc.sync.dma_start(out=st, in_=wqkv_d[kc * P:(kc + 1) * P, :])
                        nc.vector.tensor_copy(wq[:, kc, :], st)
                if nb >= LAG:
                    qk_block(0, nb - LAG, qk0, nqk0)

        # heads pools open after the transpose PSUM pool closes (bank budget)
        vt_pool = gctx.enter_context(tc.tile_pool(name="vt", bufs=2))
        hps = gctx.enter_context(tc.tile_pool(name="hps", bufs=2, space="PSUM"))
        sps = gctx.enter_context(tc.tile_pool(name="sps", bufs=2, space="PSUM"))
        tinyps = gctx.enter_context(tc.tile_pool(name="tinyps", bufs=1, space="PSUM"))

        heads_phase(0, qk0, nqk0)
        for g in range(1, G):
            qk_sb = qk_pool.tile([P, NB, 2 * GC], BF, tag="qk")
            nqk = nrmps.tile([1, 2 * GC], F32, tag="nqk")
            for nb in range(NB):
                qk_block(g, nb, qk_sb, nqk)
            heads_phase(g, qk_sb, nqk)

        gctx.close()

        # ---- Phase PROJ: y = OT^T @ Wproj + bproj (8 K=96 chunks, head-major)
        with tc.tile_pool(name="wp", bufs=1) as wp_pool, \
             tc.tile_pool(name="wpstage", bufs=2) as wpstage, \
             tc.tile_pool(name="yout", bufs=3) as yout, \
             tc.tile_pool(name="yps", bufs=2, space="PSUM") as yps:
            wp = wp_pool.tile([CH, H, C], BF)   # wp[c, h, j] = Wproj[h*CH+c, j]
            for h in range(H):
                st = wpstage.tile([CH, C], F32, tag="wpst")
                nc.sync.dma_start(out=st, in_=wproj_d[h * CH:(h + 1) * CH, :])
                nc.vector.tensor_copy(wp[:, h, :], st)

            for nb in range(NB):
                y1 = yps.tile([P, 512], F32, tag="y1")
                y2 = yps.tile([P, 256], F32, tag="y2")
                for h in range(H):
                    lhsT = ot[:, h, nb * P:(nb + 1) * P]
                    nc.tensor.matmul(y1, lhsT, wp[:, h, 0:512],
                                     start=(h == 0), stop=False)
                    nc.tensor.matmul(y2, lhsT, wp[:, h, 512:C],
                                     start=(h == 0), stop=False)
                nc.tensor.matmul(y1, ones_row, bproj_bf[0:1, 0:512],
                                 start=False, stop=True)
                nc.tensor.matmul(y2, ones_row, bproj_bf[0:1, 512:C],
                                 start=False, stop=True)
                ysb = yout.tile([P, C], F32, tag="y")
                nc.vector.tensor_copy(ysb[:, 0:512], y1)
                nc.vector.tensor_copy(ysb[:, 512:C], y2)
                nc.sync.dma_start(out=y_d[nb * P:(nb + 1) * P, :], in_=ysb)

    # Split multi-wait sync conditions into EventSemaphore instructions —
    # walrus' ACT/DVE instruction structs encode at most one wait.
    bass_rust.generate_event_semaphores(nc)
    return nc


def _in_maps(x, Wqkv, temperature, Wproj, bproj):
    wqkv = np.ascontiguousarray(Wqkv, dtype=np.float32)
    temp = np.ascontiguousarray(temperature, dtype=np.float32).reshape(H)
    wproj = np.ascontiguousarray(Wproj, dtype=np.float32)
    bp = np.ascontiguousarray(bproj, dtype=np.float32)
    return [
        {"x": np.ascontiguousarray(x[b], dtype=np.float32), "Wqkv": wqkv,
         "temperature": temp, "Wproj": wproj, "bproj": bp}
        for b in range(x.shape[0])
    ]


def run(x, Wqkv, temperature, Wproj, bproj, trace=False):
    nc = build_nc()
    in_maps = _in_maps(x, Wqkv, temperature, Wproj, bproj)
    res = run_bass_kernel_spmd(nc, in_maps, core_ids=list(range(len(in_maps))),
                               trace=trace)
    out = np.stack([res.results[b]["y"] for b in range(len(in_maps))], axis=0)
    return out.astype(np.float32), res


def kernel(x, Wqkv, temperature, Wproj, bproj):
    out, _ = run(x, Wqkv, temperature, Wproj, bproj, trace=False)
    return out


# revision 25
# speedup vs baseline: 1.3252x; 1.0343x over previous
"""Channel-attention (XCA) block on 8 trn2 NeuronCores, data-parallel over batch.

Per core: x (4096, 768) -> qkv -> per-head channel attention (96x96 scores over
l2-normalized q,k transposed to (Ch, N)) -> proj.  All big matmuls run in bf16
with fp32 PSUM accumulation; norms/softmax in fp32.

v3: group-0 qk generation fused into the x-transpose loop (PE dense from the
start, HAM stays warm), head-major attention-output layout, S-before-vT
software pipeline per head (softmax latency hidden under vT matmuls), single
eviction copies via 3D tiles.
"""

import numpy as np
from contextlib import ExitStack

import bass_rust
import concourse.bass as bass
import concourse.tile as tile
from concourse import mybir
from concourse.masks import make_identity
from concourse.bass_utils import run_bass_kernel_spmd

F32 = mybir.dt.float32
BF = mybir.dt.bfloat16
AF = mybir.ActivationFunctionType

P = 128          # partitions
N = 4096         # tokens per core (batch element)
C = 768          # channels
H = 8            # heads
CH = 96          # channels per head
KC = C // P      # 6 contraction chunks of 128
NB = N // P      # 32 token blocks of 128
N5 = N // 512    # 8 token blocks of 512
G = 4            # head groups
HPG = H // G     # 2 heads per group
GC = HPG * CH    # 192 qkv columns per group
EPS = 1e-12


def build_nc():
    nc = bass.Bass()

    x_d = nc.dram_tensor("x", [N, C], F32, kind="ExternalInput")
    wqkv_d = nc.dram_tensor("Wqkv", [C, 3 * C], F32, kind="ExternalInput")
    temp_d = nc.dram_tensor("temperature", [H], F32, kind="ExternalInput")
    wproj_d = nc.dram_tensor("Wproj", [C, C], F32, kind="ExternalInput")
    bproj_d = nc.dram_tensor("bproj", [C], F32, kind="ExternalInput")
    y_d = nc.dram_tensor("y", [N, C], F32, kind="ExternalOutput")

    with ExitStack() as ctx:
        tc = ctx.enter_context(tile.TileContext(nc))
        persist = ctx.enter_context(tc.tile_pool(name="persist", bufs=1))

        # persistent SBUF: xT[c%128, c//128, n] = x[n, c]  (bf16)
        xT = persist.tile([P, KC, N], BF)
        # Wqkv bf16: wq[c%128, c//128, j] = Wqkv[c, j]
        wq = persist.tile([P, KC, 3 * C], BF)
        # attention output, head-major: ot[c, h, n] = O[n, h*CH + c]
        ot = persist.tile([CH, H, N], BF)

        ident128 = persist.tile([P, P], F32)
        make_identity(nc, ident128)
        ident96 = persist.tile([CH, CH], BF)
        make_identity(nc, ident96)
        ones_col = persist.tile([P, 1], BF)      # norm-matmul lhsT (K=128, M=1)
        nc.vector.memset(ones_col, 1.0)
        ones_row = persist.tile([1, P], BF)      # bias-matmul lhsT (K=1, M=128)
        nc.vector.memset(ones_row, 1.0)
        one1 = persist.tile([1, 1], F32)         # row->col matmul rhs
        nc.vector.memset(one1, 1.0)
        ones96 = persist.tile([1, CH], F32)
        nc.vector.memset(ones96, 1.0)

        temp_sb = persist.tile([1, H], F32)
        bstage = persist.tile([1, C], F32)
        bstage_bf = persist.tile([1, C], BF)

        gctx = ctx.enter_context(ExitStack())
        qk_pool = gctx.enter_context(tc.tile_pool(name="qk", bufs=1))
        qkps = gctx.enter_context(tc.tile_pool(name="qkps", bufs=2, space="PSUM"))
        nrmps = gctx.enter_context(tc.tile_pool(name="nrmps", bufs=1, space="PSUM"))
        sqpool = gctx.enter_context(tc.tile_pool(name="sq", bufs=3))
        small = gctx.enter_context(tc.tile_pool(name="small", bufs=2))

        def qk_block(g, nb, qk_sb, nqk):
            """qkv q|k matmuls for one token block + eviction + norm matmul."""
            qkp = qkps.tile([P, 2 * GC], F32, tag="qkp")
            qp = qkp[:, 0:GC]
            kp = qkp[:, GC:2 * GC]
            # qp|kp live in one PSUM bank: one accumulation group (start
            # pending-zeroes the whole bank, k region accumulates onto 0)
            for kc in range(KC):
                lhsT = xT[:, kc, nb * P:(nb + 1) * P]
                nc.tensor.matmul(
                    qp, lhsT, wq[:, kc, g * GC:(g + 1) * GC],
                    start=(kc == 0), stop=False)
                nc.tensor.matmul(
                    kp, lhsT, wq[:, kc, C + g * GC: C + (g + 1) * GC],
                    start=False, stop=(kc == KC - 1))
            qks = qk_sb[:, nb, :]
            nc.vector.tensor_copy(qks, qkp)
            sq = sqpool.tile([P, 2 * GC], BF, tag="sq")
            nc.scalar.activation(sq, qks, AF.Square)
            nc.tensor.matmul(nqk, ones_col, sq,
                             start=(nb == 0), stop=(nb == NB - 1))

        def heads_phase(g, qk_sb, nqk):
            """scores, softmax (hidden under vT gen), out tiles for one group."""
            # scores first: PE stays busy while the norm chain completes
            s_list = []
            for hh in range(HPG):
                s_ps = sps.tile([CH, CH], F32, tag="s")
                for nb in range(NB):
                    nc.tensor.matmul(
                        s_ps,
                        qk_sb[:, nb, hh * CH:(hh + 1) * CH],
                        qk_sb[:, nb, GC + hh * CH: GC + (hh + 1) * CH],
                        start=(nb == 0), stop=(nb == NB - 1))
                s_list.append(s_ps)

            # rinv row: 1 / max(sqrt(sumsq), eps), [q | k] in one (1, 384) row
            rqk = small.tile([1, 2 * GC], F32, tag="rqk")
            nc.scalar.activation(rqk, nqk, AF.Sqrt)
            nc.vector.tensor_scalar_max(rqk, rqk, EPS)
            nc.vector.reciprocal(rqk, rqk)

            # per-head norm-derived tiles (tiny PE matmuls, off the pipeline)
            rq_cols, r_sbs = [], []
            for hh in range(HPG):
                h = g * HPG + hh
                rq_ps = tinyps.tile([CH, 1], F32, tag="tp")
                nc.tensor.matmul(rq_ps, rqk[0:1, hh * CH:(hh + 1) * CH], one1,
                                 start=True, stop=True)
                rq_col = small.tile([CH, 1], F32, tag="rqc")
                nc.vector.tensor_copy(rq_col, rq_ps)
                tempb = small.tile([1, CH], F32, tag="tb")
                nc.scalar.activation(tempb, ones96, AF.Copy,
                                     scale=temp_sb[0:1, h:h + 1])
                r_ps = tinyps.tile([CH, CH], F32, tag="tp")
                nc.tensor.matmul(r_ps, tempb,
                                 rqk[0:1, GC + hh * CH: GC + (hh + 1) * CH],
                                 start=True, stop=True)
                r_sb = small.tile([CH, CH], F32, tag="rsb")
                nc.vector.tensor_copy(r_sb, r_ps)
                rq_cols.append(rq_col)
                r_sbs.append(r_sb)

            for hh in range(HPG):
                h = g * HPG + hh
                # softmax chain on DVE/ACT — overlaps the vT matmuls below
                z_sb = small.tile([CH, CH], F32, tag="z")
                nc.vector.tensor_mul(z_sb, s_list[hh], r_sbs[hh])
                e_sb = small.tile([CH, CH], BF, tag="e")
                sume = small.tile([CH, 1], F32, tag="se")
                nc.scalar.activation(e_sb, z_sb, AF.Exp,
                                     scale=rq_cols[hh], accum_out=sume)
                rden = small.tile([CH, 1], F32, tag="rd")
                nc.vector.reciprocal(rden, sume)
                attn_s = small.tile([CH, CH], BF, tag="at")
                nc.scalar.activation(attn_s, e_sb, AF.Copy, scale=rden)

                # vT for this head: (96 d, 4096 n) bf16, from Wqkv v-cols and xT
                vt_sb = vt_pool.tile([CH, N], BF, tag="vt")
                for n5 in range(N5):
                    vp = hps.tile([CH, 512], F32, tag="hp")
                    for kc in range(KC):
                        nc.tensor.matmul(
                            vp, wq[:, kc, 2 * C + h * CH: 2 * C + (h + 1) * CH],
                            xT[:, kc, n5 * 512:(n5 + 1) * 512],
                            start=(kc == 0), stop=(kc == KC - 1))
                    nc.vector.tensor_copy(vt_sb[:, n5 * 512:(n5 + 1) * 512], vp)

                # attn^T via PE transpose (ready well before vT finishes)
                et_ps = tinyps.tile([CH, CH], BF, tag="tp")
                nc.tensor.transpose(et_ps, attn_s, ident96)
                et_sb = small.tile([CH, CH], BF, tag="et")
                nc.vector.tensor_copy(et_sb, et_ps)

                # out_h = attn @ vT -> ot[:, h, n] (single copy per tile)
                for n5 in range(N5):
                    op_ = hps.tile([CH, 512], F32, tag="hp")
                    nc.tensor.matmul(op_, et_sb, vt_sb[:, n5 * 512:(n5 + 1) * 512],
                                     start=True, stop=True)
                    nc.vector.tensor_copy(ot[:, h, n5 * 512:(n5 + 1) * 512], op_)

        # ---- Phase T fused with group-0 qk: x -> xT + q/k(g0), PE dense ----
        # x loads start before the 7MB Wqkv load so the transposes (and HAM
        # warmup) begin immediately; qk blocks trail by 3 iterations.
        LAG = 3
        qk0 = qk_pool.tile([P, NB, 2 * GC], BF, tag="qk")
        nqk0 = nrmps.tile([1, 2 * GC], F32, tag="nqk")
        with tc.tile_pool(name="xstage", bufs=4) as xstage, \
             tc.tile_pool(name="wstage", bufs=2) as wstage, \
             tc.tile_pool(name="tps", bufs=2, space="PSUM") as tps:
            for nb in range(NB + LAG):
                if nb < NB:
                    xt_ = xstage.tile([P, C], F32, tag="x")
                    nc.sync.dma_start(out=xt_, in_=x_d[nb * P:(nb + 1) * P, :])
                    tall = tps.tile([P, KC, P], F32, tag="t")  # banks: kc 0-3|4-5
                    for kc in range(KC):
                        nc.tensor.matmul(tall[:, kc, :], xt_[:, kc * P:(kc + 1) * P],
                                         ident128, is_transpose=True,
                                         start=(kc in (0, 4)), stop=(kc in (3, 5)))
                    nc.vector.tensor_copy(xT[:, :, nb * P:(nb + 1) * P], tall)
                if nb == LAG - 1:
                    for kc in range(KC):
                        st = wstage.tile([P, 3 * C], F32, tag="wst")
                        nc.sync.dma_start(out=st, in_=wqkv_d[kc * P:(kc + 1) * P, :])
                        nc.vector.tensor_copy(wq[:, kc, :], st)
                    # small loads, behind the bulk weights on the queue
                    nc.sync.dma_start(out=temp_sb,
                                      in_=temp_d.rearrange("(a h) -> a h", a=1))
                    nc.sync.dma_start(out=bstage,
                                      in_=bproj_d.rearrange("(a c) -> a c", a=1))
                    nc.vector.tensor_copy(bstage_bf, bstage)
                if nb >= LAG:
                    qk_block(0, nb - LAG, qk0, nqk0)

        # heads pools open after the transpose PSUM pool closes (bank budget)
        vt_pool = gctx.enter_context(tc.tile_pool(name="vt", bufs=2))
        hps = gctx.enter_context(tc.tile_pool(name="hps", bufs=2, space="PSUM"))
        sps = gctx.enter_context(tc.tile_pool(name="sps", bufs=2, space="PSUM"))
        tinyps = gctx.enter_context(tc.tile_pool(name="tinyps", bufs=1, space="PSUM"))

        heads_phase(0, qk0, nqk0)
        for g in range(1, G):
            qk_sb = qk_pool.tile([P, NB, 2 * GC], BF, tag="qk")
            nqk = nrmps.tile([1, 2 * GC], F32, tag="nqk")
            for nb in range(NB):
                qk_block(g, nb, qk_sb, nqk)
            heads_phase(g, qk_sb, nqk)

        gctx.close()

        # ---- Phase PROJ: y = OT^T @ Wproj + bproj (8 K=96 chunks, head-major)
        with tc.tile_pool(name="wp", bufs=1) as wp_pool, \
             tc.tile_pool(name="wpstage", bufs=2) as wpstage, \
             tc.tile_pool(name="yout", bufs=3) as yout, \
             tc.tile_pool(name="yps", bufs=2, space="PSUM") as yps:
            wp = wp_pool.tile([CH, H, C], BF)   # wp[c, h, j] = Wproj[h*CH+c, j]
            for h in range(H):
                st = wpstage.tile([CH, C], F32, tag="wpst")
                nc.sync.dma_start(out=st, in_=wproj_d[h * CH:(h + 1) * CH, :])
                nc.vector.tensor_copy(wp[:, h, :], st)

            # bias broadcast to all 128 rows via K=1 matmul, once; then the
            # PROJ eviction is an add instead of a copy (no per-block bias MMs)
            bias_sb = wp_pool.tile([P, C], F32)
            for (a, b) in ((0, 512), (512, C)):
                bps = yps.tile([P, b - a], F32, tag="y1")
                nc.tensor.matmul(bps, ones_row, bstage_bf[0:1, a:b],
                                 start=True, stop=True)
                nc.vector.tensor_copy(bias_sb[:, a:b], bps)

            for nb in range(NB):
                y1 = yps.tile([P, 512], F32, tag="y1")
                y2 = yps.tile([P, 256], F32, tag="y2")
                for h in range(H):
                    lhsT = ot[:, h, nb * P:(nb + 1) * P]
                    nc.tensor.matmul(y1, lhsT, wp[:, h, 0:512],
                                     start=(h == 0), stop=(h == H - 1))
                    nc.tensor.matmul(y2, lhsT, wp[:, h, 512:C],
                                     start=(h == 0), stop=(h == H - 1))
                ysb = yout.tile([P, C], F32, tag="y")
                nc.vector.tensor_add(ysb[:, 0:512], y1, bias_sb[:, 0:512])
                nc.vector.tensor_add(ysb[:, 512:C], y2, bias_sb[:, 512:C])
                nc.sync.dma_start(out=y_d[nb * P:(nb + 1) * P, :], in_=ysb)

    # Split multi-wait sync conditions into EventSemaphore instructions —
    # walrus' ACT/DVE instruction structs encode at most one wait.
    bass_rust.generate_event_semaphores(nc)
    return nc


def _in_maps(x, Wqkv, temperature, Wproj, bproj):
    wqkv = np.ascontiguousarray(Wqkv, dtype=np.float32)
    temp = np.ascontiguousarray(temperature, dtype=np.float32).reshape(H)
    wproj = np.ascontiguousarray(Wproj, dtype=np.float32)
    bp = np.ascontiguousarray(bproj, dtype=np.float32)
    return [
        {"x": np.ascontiguousarray(x[b], dtype=np.float32), "Wqkv": wqkv,
         "temperature": temp, "Wproj": wproj, "bproj": bp}
        for b in range(x.shape[0])
    ]


def run(x, Wqkv, temperature, Wproj, bproj, trace=False):
    nc = build_nc()
    in_maps = _in_maps(x, Wqkv, temperature, Wproj, bproj)
    res = run_bass_kernel_spmd(nc, in_maps, core_ids=list(range(len(in_maps))),
                               trace=trace)
    out = np.stack([res.results[b]["y"] for b in range(len(in_maps))], axis=0)
    return out.astype(np.float32), res


def kernel(x, Wqkv, temperature, Wproj, bproj):
    out, _ = run(x, Wqkv, temperature, Wproj, bproj, trace=False)
    return out


# revision 28
# speedup vs baseline: 1.3360x; 1.0082x over previous
"""Channel-attention (XCA) block on 8 trn2 NeuronCores, data-parallel over batch.

Per core: x (4096, 768) -> qkv -> per-head channel attention (96x96 scores over
l2-normalized q,k transposed to (Ch, N)) -> proj.  All big matmuls run in bf16
with fp32 PSUM accumulation; norms/softmax in fp32.

v3: group-0 qk generation fused into the x-transpose loop (PE dense from the
start, HAM stays warm), head-major attention-output layout, S-before-vT
software pipeline per head (softmax latency hidden under vT matmuls), single
eviction copies via 3D tiles.
"""

import numpy as np
from contextlib import ExitStack

import bass_rust
import concourse.bass as bass
import concourse.tile as tile
from concourse import mybir
from concourse.masks import make_identity
from concourse.bass_utils import run_bass_kernel_spmd

F32 = mybir.dt.float32
BF = mybir.dt.bfloat16
AF = mybir.ActivationFunctionType

P = 128          # partitions
N = 4096         # tokens per core (batch element)
C = 768          # channels
H = 8            # heads
CH = 96          # channels per head
KC = C // P      # 6 contraction chunks of 128
NB = N // P      # 32 token blocks of 128
N5 = N // 512    # 8 token blocks of 512
G = 4            # head groups
HPG = H // G     # 2 heads per group
GC = HPG * CH    # 192 qkv columns per group
EPS = 1e-12


def build_nc():
    nc = bass.Bass()

    x_d = nc.dram_tensor("x", [N, C], F32, kind="ExternalInput")
    wqkv_d = nc.dram_tensor("Wqkv", [C, 3 * C], F32, kind="ExternalInput")
    temp_d = nc.dram_tensor("temperature", [H], F32, kind="ExternalInput")
    wproj_d = nc.dram_tensor("Wproj", [C, C], F32, kind="ExternalInput")
    bproj_d = nc.dram_tensor("bproj", [C], F32, kind="ExternalInput")
    y_d = nc.dram_tensor("y", [N, C], F32, kind="ExternalOutput")

    with ExitStack() as ctx:
        tc = ctx.enter_context(tile.TileContext(nc))
        persist = ctx.enter_context(tc.tile_pool(name="persist", bufs=1))

        # persistent SBUF: xT[c%128, c//128, n] = x[n, c]  (bf16)
        xT = persist.tile([P, KC, N], BF)
        # Wqkv bf16: wq[c%128, c//128, j] = Wqkv[c, j]
        wq = persist.tile([P, KC, 3 * C], BF)
        # attention output, head-major: ot[c, h, n] = O[n, h*CH + c]
        ot = persist.tile([CH, H, N], BF)

        ident128 = persist.tile([P, P], F32)
        make_identity(nc, ident128)
        ident96 = persist.tile([CH, CH], BF)
        make_identity(nc, ident96)
        ones_col = persist.tile([P, 1], BF)      # norm-matmul lhsT (K=128, M=1)
        nc.vector.memset(ones_col, 1.0)
        ones_row = persist.tile([1, P], BF)      # bias-matmul lhsT (K=1, M=128)
        nc.vector.memset(ones_row, 1.0)
        one1 = persist.tile([1, 1], F32)         # row->col matmul rhs
        nc.vector.memset(one1, 1.0)
        ones96 = persist.tile([1, CH], F32)
        nc.vector.memset(ones96, 1.0)

        temp_sb = persist.tile([1, H], F32)
        bstage = persist.tile([1, C], F32)
        bstage_bf = persist.tile([1, C], BF)

        gctx = ctx.enter_context(ExitStack())
        qk_pool = gctx.enter_context(tc.tile_pool(name="qk", bufs=1))
        qkps = gctx.enter_context(tc.tile_pool(name="qkps", bufs=2, space="PSUM"))
        nrmps = gctx.enter_context(tc.tile_pool(name="nrmps", bufs=1, space="PSUM"))
        sqpool = gctx.enter_context(tc.tile_pool(name="sq", bufs=3))
        small = gctx.enter_context(tc.tile_pool(name="small", bufs=2))

        def qk_block(g, nb, qk_sb, nqk):
            """qkv q|k matmuls for one token block + eviction + norm matmul."""
            qkp = qkps.tile([P, 2 * GC], F32, tag="qkp")
            qp = qkp[:, 0:GC]
            kp = qkp[:, GC:2 * GC]
            # qp|kp live in one PSUM bank: one accumulation group (start
            # pending-zeroes the whole bank, k region accumulates onto 0)
            for kc in range(KC):
                lhsT = xT[:, kc, nb * P:(nb + 1) * P]
                nc.tensor.matmul(
                    qp, lhsT, wq[:, kc, g * GC:(g + 1) * GC],
                    start=(kc == 0), stop=False)
                nc.tensor.matmul(
                    kp, lhsT, wq[:, kc, C + g * GC: C + (g + 1) * GC],
                    start=False, stop=(kc == KC - 1))
            qks = qk_sb[:, nb, :]
            nc.vector.tensor_copy(qks, qkp)
            sq = sqpool.tile([P, 2 * GC], BF, tag="sq")
            nc.scalar.activation(sq, qks, AF.Square)
            nc.tensor.matmul(nqk, ones_col, sq,
                             start=(nb == 0), stop=(nb == NB - 1))

        def heads_phase(g, qk_sb, nqk):
            """scores, softmax (hidden under vT gen), out tiles for one group."""
            # scores first: PE stays busy while the norm chain completes
            s_list = []
            for hh in range(HPG):
                s_ps = sps.tile([CH, CH], F32, tag="s")
                for nb in range(NB):
                    nc.tensor.matmul(
                        s_ps,
                        qk_sb[:, nb, hh * CH:(hh + 1) * CH],
                        qk_sb[:, nb, GC + hh * CH: GC + (hh + 1) * CH],
                        start=(nb == 0), stop=(nb == NB - 1))
                s_list.append(s_ps)

            # rinv row: 1 / max(sqrt(sumsq), eps), [q | k] in one (1, 384) row
            rqk = small.tile([1, 2 * GC], F32, tag="rqk")
            nc.scalar.activation(rqk, nqk, AF.Sqrt)
            nc.vector.tensor_scalar_max(rqk, rqk, EPS)
            nc.vector.reciprocal(rqk, rqk)

            # per-head norm-derived tiles (tiny PE matmuls, off the pipeline)
            rq_cols, r_sbs = [], []
            for hh in range(HPG):
                h = g * HPG + hh
                rq_ps = tinyps.tile([CH, 1], F32, tag="tp")
                nc.tensor.matmul(rq_ps, rqk[0:1, hh * CH:(hh + 1) * CH], one1,
                                 start=True, stop=True)
                rq_col = small.tile([CH, 1], F32, tag="rqc")
                nc.vector.tensor_copy(rq_col, rq_ps)
                tempb = small.tile([1, CH], F32, tag="tb")
                nc.scalar.activation(tempb, ones96, AF.Copy,
                                     scale=temp_sb[0:1, h:h + 1])
                r_ps = tinyps.tile([CH, CH], F32, tag="tp")
                nc.tensor.matmul(r_ps, tempb,
                                 rqk[0:1, GC + hh * CH: GC + (hh + 1) * CH],
                                 start=True, stop=True)
                r_sb = small.tile([CH, CH], F32, tag="rsb")
                nc.vector.tensor_copy(r_sb, r_ps)
                rq_cols.append(rq_col)
                r_sbs.append(r_sb)

            for hh in range(HPG):
                h = g * HPG + hh
                # softmax chain on DVE/ACT — overlaps the vT matmuls below
                z_sb = small.tile([CH, CH], F32, tag="z")
                nc.vector.tensor_mul(z_sb, s_list[hh], r_sbs[hh])
                e_sb = small.tile([CH, CH], BF, tag="e")
                sume = small.tile([CH, 1], F32, tag="se")
                nc.scalar.activation(e_sb, z_sb, AF.Exp,
                                     scale=rq_cols[hh], accum_out=sume)
                rden = small.tile([CH, 1], F32, tag="rd")
                nc.vector.reciprocal(rden, sume)
                attn_s = small.tile([CH, CH], BF, tag="at")
                nc.scalar.activation(attn_s, e_sb, AF.Copy, scale=rden)

                # vT for this head: (96 d, 4096 n) bf16, from Wqkv v-cols and xT
                vt_sb = vt_pool.tile([CH, N], BF, tag="vt")
                for n5 in range(N5):
                    vp = hps.tile([CH, 512], F32, tag="hp")
                    for kc in range(KC):
                        nc.tensor.matmul(
                            vp, wq[:, kc, 2 * C + h * CH: 2 * C + (h + 1) * CH],
                            xT[:, kc, n5 * 512:(n5 + 1) * 512],
                            start=(kc == 0), stop=(kc == KC - 1))
                    nc.vector.tensor_copy(vt_sb[:, n5 * 512:(n5 + 1) * 512], vp)

                # attn^T via PE transpose (ready well before vT finishes)
                et_ps = tinyps.tile([CH, CH], BF, tag="tp")
                nc.tensor.transpose(et_ps, attn_s, ident96)
                et_sb = small.tile([CH, CH], BF, tag="et")
                nc.vector.tensor_copy(et_sb, et_ps)

                # out_h = attn @ vT -> ot[:, h, n] (single copy per tile)
                for n5 in range(N5):
                    op_ = hps.tile([CH, 512], F32, tag="hp")
                    nc.tensor.matmul(op_, et_sb, vt_sb[:, n5 * 512:(n5 + 1) * 512],
                                     start=True, stop=True)
                    nc.vector.tensor_copy(ot[:, h, n5 * 512:(n5 + 1) * 512], op_)

        # ---- Phase T fused with group-0 qk: x -> xT + q/k(g0), PE dense ----
        # x loads start before the 7MB Wqkv load so the transposes (and HAM
        # warmup) begin immediately; qk blocks trail by 3 iterations.
        LAG = KC + 1
        qk0 = qk_pool.tile([P, NB, 2 * GC], BF, tag="qk")
        nqk0 = nrmps.tile([1, 2 * GC], F32, tag="nqk")
        with tc.tile_pool(name="xstage", bufs=4) as xstage, \
             tc.tile_pool(name="wstage", bufs=2) as wstage, \
             tc.tile_pool(name="tps", bufs=2, space="PSUM") as tps:
            for nb in range(NB + LAG):
                if nb < NB:
                    xt_ = xstage.tile([P, C], F32, tag="x")
                    nc.sync.dma_start(out=xt_, in_=x_d[nb * P:(nb + 1) * P, :])
                    tall = tps.tile([P, KC, P], F32, tag="t")  # banks: kc 0-3|4-5
                    for kc in range(KC):
                        nc.tensor.matmul(tall[:, kc, :], xt_[:, kc * P:(kc + 1) * P],
                                         ident128, is_transpose=True,
                                         start=(kc in (0, 4)), stop=(kc in (3, 5)))
                    nc.vector.tensor_copy(xT[:, :, nb * P:(nb + 1) * P], tall)
                if 1 <= nb <= KC:
                    # one Wqkv chunk per iteration, interleaved with x loads so
                    # neither stream queues fully behind the other; the q|k
                    # half is cast separately so qk matmuls unblock early
                    kc = nb - 1
                    half = 3 * C // 2
                    st = wstage.tile([P, 3 * C], F32, tag="wst")
                    nc.sync.dma_start(out=st[:, 0:half],
                                      in_=wqkv_d[kc * P:(kc + 1) * P, 0:half])
                    nc.vector.tensor_copy(wq[:, kc, 0:half], st[:, 0:half])
                    nc.sync.dma_start(out=st[:, half:],
                                      in_=wqkv_d[kc * P:(kc + 1) * P, half:])
                    nc.vector.tensor_copy(wq[:, kc, half:], st[:, half:])
                if nb == KC + 1:
                    nc.sync.dma_start(out=temp_sb,
                                      in_=temp_d.rearrange("(a h) -> a h", a=1))
                    nc.sync.dma_start(out=bstage,
                                      in_=bproj_d.rearrange("(a c) -> a c", a=1))
                    nc.vector.tensor_copy(bstage_bf, bstage)
                if nb >= LAG:
                    qk_block(0, nb - LAG, qk0, nqk0)

        # heads pools open after the transpose PSUM pool closes (bank budget)
        vt_pool = gctx.enter_context(tc.tile_pool(name="vt", bufs=2))
        hps = gctx.enter_context(tc.tile_pool(name="hps", bufs=2, space="PSUM"))
        sps = gctx.enter_context(tc.tile_pool(name="sps", bufs=2, space="PSUM"))
        tinyps = gctx.enter_context(tc.tile_pool(name="tinyps", bufs=1, space="PSUM"))

        heads_phase(0, qk0, nqk0)
        for g in range(1, G):
            qk_sb = qk_pool.tile([P, NB, 2 * GC], BF, tag="qk")
            nqk = nrmps.tile([1, 2 * GC], F32, tag="nqk")
            for nb in range(NB):
                qk_block(g, nb, qk_sb, nqk)
            heads_phase(g, qk_sb, nqk)

        gctx.close()

        # ---- Phase PROJ: y = OT^T @ Wproj + bproj (8 K=96 chunks, head-major)
        with tc.tile_pool(name="wp", bufs=1) as wp_pool, \
             tc.tile_pool(name="wpstage", bufs=2) as wpstage, \
             tc.tile_pool(name="yout", bufs=3) as yout, \
             tc.tile_pool(name="yps", bufs=2, space="PSUM") as yps:
            wp = wp_pool.tile([CH, H, C], BF)   # wp[c, h, j] = Wproj[h*CH+c, j]
            for h in range(H):
                st = wpstage.tile([CH, C], F32, tag="wpst")
                nc.sync.dma_start(out=st, in_=wproj_d[h * CH:(h + 1) * CH, :])
                nc.vector.tensor_copy(wp[:, h, :], st)

            # bias broadcast to all 128 rows via K=1 matmul, once; then the
            # PROJ eviction is an add instead of a copy (no per-block bias MMs)
            bias_sb = wp_pool.tile([P, C], F32)
            for (a, b) in ((0, 512), (512, C)):
                bps = yps.tile([P, b - a], F32, tag="y1")
                nc.tensor.matmul(bps, ones_row, bstage_bf[0:1, a:b],
                                 start=True, stop=True)
                nc.vector.tensor_copy(bias_sb[:, a:b], bps)

            for nb in range(NB):
                y1 = yps.tile([P, 512], F32, tag="y1")
                y2 = yps.tile([P, 256], F32, tag="y2")
                for h in range(H):
                    lhsT = ot[:, h, nb * P:(nb + 1) * P]
                    nc.tensor.matmul(y1, lhsT, wp[:, h, 0:512],
                                     start=(h == 0), stop=(h == H - 1))
                    nc.tensor.matmul(y2, lhsT, wp[:, h, 512:C],
                                     start=(h == 0), stop=(h == H - 1))
                ysb = yout.tile([P, C], F32, tag="y")
                nc.vector.tensor_add(ysb[:, 0:512], y1, bias_sb[:, 0:512])
                nc.vector.tensor_add(ysb[:, 512:C], y2, bias_sb[:, 512:C])
                nc.sync.dma_start(out=y_d[nb * P:(nb + 1) * P, :], in_=ysb)

    # Split multi-wait sync conditions into EventSemaphore instructions —
    # walrus' ACT/DVE instruction structs encode at most one wait.
    bass_rust.generate_event_semaphores(nc)
    return nc


def _in_maps(x, Wqkv, temperature, Wproj, bproj):
    wqkv = np.ascontiguousarray(Wqkv, dtype=np.float32)
    temp = np.ascontiguousarray(temperature, dtype=np.float32).reshape(H)
    wproj = np.ascontiguousarray(Wproj, dtype=np.float32)
    bp = np.ascontiguousarray(bproj, dtype=np.float32)
    return [
        {"x": np.ascontiguousarray(x[b], dtype=np.float32), "Wqkv": wqkv,
         "temperature": temp, "Wproj": wproj, "bproj": bp}
        for b in range(x.shape[0])
    ]


def run(x, Wqkv, temperature, Wproj, bproj, trace=False):
    nc = build_nc()
    in_maps = _in_maps(x, Wqkv, temperature, Wproj, bproj)
    res = run_bass_kernel_spmd(nc, in_maps, core_ids=list(range(len(in_maps))),
                               trace=trace)
    out = np.stack([res.results[b]["y"] for b in range(len(in_maps))], axis=0)
    return out.astype(np.float32), res


def kernel(x, Wqkv, temperature, Wproj, bproj):
    out, _ = run(x, Wqkv, temperature, Wproj, bproj, trace=False)
    return out


# revision 29
# speedup vs baseline: 1.3423x; 1.0047x over previous
"""Channel-attention (XCA) block on 8 trn2 NeuronCores, data-parallel over batch.

Per core: x (4096, 768) -> qkv -> per-head channel attention (96x96 scores over
l2-normalized q,k transposed to (Ch, N)) -> proj.  All big matmuls run in bf16
with fp32 PSUM accumulation; norms/softmax in fp32.

v3: group-0 qk generation fused into the x-transpose loop (PE dense from the
start, HAM stays warm), head-major attention-output layout, S-before-vT
software pipeline per head (softmax latency hidden under vT matmuls), single
eviction copies via 3D tiles.
"""

import numpy as np
from contextlib import ExitStack

import bass_rust
import concourse.bass as bass
import concourse.tile as tile
from concourse import mybir
from concourse.masks import make_identity
from concourse.bass_utils import run_bass_kernel_spmd

F32 = mybir.dt.float32
BF = mybir.dt.bfloat16
AF = mybir.ActivationFunctionType

P = 128          # partitions
N = 4096         # tokens per core (batch element)
C = 768          # channels
H = 8            # heads
CH = 96          # channels per head
KC = C // P      # 6 contraction chunks of 128
NB = N // P      # 32 token blocks of 128
N5 = N // 512    # 8 token blocks of 512
G = 4            # head groups
HPG = H // G     # 2 heads per group
GC = HPG * CH    # 192 qkv columns per group
EPS = 1e-12


def build_nc():
    nc = bass.Bass()

    x_d = nc.dram_tensor("x", [N, C], F32, kind="ExternalInput")
    wqkv_d = nc.dram_tensor("Wqkv", [C, 3 * C], F32, kind="ExternalInput")
    temp_d = nc.dram_tensor("temperature", [H], F32, kind="ExternalInput")
    wproj_d = nc.dram_tensor("Wproj", [C, C], F32, kind="ExternalInput")
    bproj_d = nc.dram_tensor("bproj", [C], F32, kind="ExternalInput")
    y_d = nc.dram_tensor("y", [N, C], F32, kind="ExternalOutput")

    with ExitStack() as ctx:
        tc = ctx.enter_context(tile.TileContext(nc))
        persist = ctx.enter_context(tc.tile_pool(name="persist", bufs=1))

        # persistent SBUF: xT[c%128, c//128, n] = x[n, c]  (bf16)
        xT = persist.tile([P, KC, N], BF)
        # Wqkv bf16: wq[c%128, c//128, j] = Wqkv[c, j]
        wq = persist.tile([P, KC, 3 * C], BF)
        # attention output, head-major: ot[c, h, n] = O[n, h*CH + c]
        ot = persist.tile([CH, H, N], BF)

        ident128 = persist.tile([P, P], F32)
        make_identity(nc, ident128)
        ident96 = persist.tile([CH, CH], BF)
        make_identity(nc, ident96)
        ones_col = persist.tile([P, 1], BF)      # norm-matmul lhsT (K=128, M=1)
        nc.vector.memset(ones_col, 1.0)
        ones_row = persist.tile([1, P], BF)      # bias-matmul lhsT (K=1, M=128)
        nc.vector.memset(ones_row, 1.0)
        one1 = persist.tile([1, 1], F32)         # row->col matmul rhs
        nc.vector.memset(one1, 1.0)
        ones96 = persist.tile([1, CH], F32)
        nc.vector.memset(ones96, 1.0)

        temp_sb = persist.tile([1, H], F32)
        bstage = persist.tile([1, C], F32)
        bstage_bf = persist.tile([1, C], BF)

        gctx = ctx.enter_context(ExitStack())
        qk_pool = gctx.enter_context(tc.tile_pool(name="qk", bufs=1))
        qkps = gctx.enter_context(tc.tile_pool(name="qkps", bufs=2, space="PSUM"))
        nrmps = gctx.enter_context(tc.tile_pool(name="nrmps", bufs=1, space="PSUM"))
        sqpool = gctx.enter_context(tc.tile_pool(name="sq", bufs=3))
        small = gctx.enter_context(tc.tile_pool(name="small", bufs=2))

        def qk_block(g, nb, qk_sb, nqk):
            """qkv q|k matmuls for one token block + eviction + norm matmul."""
            qkp = qkps.tile([P, 2 * GC], F32, tag="qkp")
            qp = qkp[:, 0:GC]
            kp = qkp[:, GC:2 * GC]
            # qp|kp live in one PSUM bank: one accumulation group (start
            # pending-zeroes the whole bank, k region accumulates onto 0)
            for kc in range(KC):
                lhsT = xT[:, kc, nb * P:(nb + 1) * P]
                nc.tensor.matmul(
                    qp, lhsT, wq[:, kc, g * GC:(g + 1) * GC],
                    start=(kc == 0), stop=False)
                nc.tensor.matmul(
                    kp, lhsT, wq[:, kc, C + g * GC: C + (g + 1) * GC],
                    start=False, stop=(kc == KC - 1))
            qks = qk_sb[:, nb, :]
            nc.vector.tensor_copy(qks, qkp)
            sq = sqpool.tile([P, 2 * GC], BF, tag="sq")
            nc.scalar.activation(sq, qks, AF.Square)
            nc.tensor.matmul(nqk, ones_col, sq,
                             start=(nb == 0), stop=(nb == NB - 1))

        def heads_phase(g, qk_sb, nqk):
            """scores, softmax (hidden under vT gen), out tiles for one group."""
            # scores first: PE stays busy while the norm chain completes
            s_list = []
            for hh in range(HPG):
                s_ps = sps.tile([CH, CH], F32, tag="s")
                for nb in range(NB):
                    nc.tensor.matmul(
                        s_ps,
                        qk_sb[:, nb, hh * CH:(hh + 1) * CH],
                        qk_sb[:, nb, GC + hh * CH: GC + (hh + 1) * CH],
                        start=(nb == 0), stop=(nb == NB - 1))
                s_list.append(s_ps)

            # rinv row: 1 / max(sqrt(sumsq), eps), [q | k] in one (1, 384) row
            rqk = small.tile([1, 2 * GC], F32, tag="rqk")
            nc.scalar.activation(rqk, nqk, AF.Sqrt)
            nc.vector.tensor_scalar_max(rqk, rqk, EPS)
            nc.vector.reciprocal(rqk, rqk)

            # per-head norm-derived tiles (tiny PE matmuls, off the pipeline)
            rq_cols, r_sbs = [], []
            for hh in range(HPG):
                h = g * HPG + hh
                rq_ps = tinyps.tile([CH, 1], F32, tag="tp")
                nc.tensor.matmul(rq_ps, rqk[0:1, hh * CH:(hh + 1) * CH], one1,
                                 start=True, stop=True)
                rq_col = small.tile([CH, 1], F32, tag="rqc")
                nc.vector.tensor_copy(rq_col, rq_ps)
                tempb = small.tile([1, CH], F32, tag="tb")
                nc.scalar.activation(tempb, ones96, AF.Copy,
                                     scale=temp_sb[0:1, h:h + 1])
                r_ps = tinyps.tile([CH, CH], F32, tag="tp")
                nc.tensor.matmul(r_ps, tempb,
                                 rqk[0:1, GC + hh * CH: GC + (hh + 1) * CH],
                                 start=True, stop=True)
                r_sb = small.tile([CH, CH], F32, tag="rsb")
                nc.vector.tensor_copy(r_sb, r_ps)
                rq_cols.append(rq_col)
                r_sbs.append(r_sb)

            for hh in range(HPG):
                h = g * HPG + hh
                # softmax chain on DVE/ACT — overlaps the vT matmuls below
                z_sb = small.tile([CH, CH], F32, tag="z")
                nc.vector.tensor_mul(z_sb, s_list[hh], r_sbs[hh])
                e_sb = small.tile([CH, CH], BF, tag="e")
                sume = small.tile([CH, 1], F32, tag="se")
                nc.scalar.activation(e_sb, z_sb, AF.Exp,
                                     scale=rq_cols[hh], accum_out=sume)
                rden = small.tile([CH, 1], F32, tag="rd")
                nc.vector.reciprocal(rden, sume)
                attn_s = small.tile([CH, CH], BF, tag="at")
                nc.scalar.activation(attn_s, e_sb, AF.Copy, scale=rden)

                # vT for this head: (96 d, 4096 n) bf16, from Wqkv v-cols and xT
                vt_sb = vt_pool.tile([CH, N], BF, tag="vt")
                for n5 in range(N5):
                    vp = hps.tile([CH, 512], F32, tag="hp")
                    for kc in range(KC):
                        nc.tensor.matmul(
                            vp, wq[:, kc, 2 * C + h * CH: 2 * C + (h + 1) * CH],
                            xT[:, kc, n5 * 512:(n5 + 1) * 512],
                            start=(kc == 0), stop=(kc == KC - 1))
                    nc.vector.tensor_copy(vt_sb[:, n5 * 512:(n5 + 1) * 512], vp)

                # attn^T via PE transpose (ready well before vT finishes)
                et_ps = tinyps.tile([CH, CH], BF, tag="tp")
                nc.tensor.transpose(et_ps, attn_s, ident96)
                et_sb = small.tile([CH, CH], BF, tag="et")
                nc.vector.tensor_copy(et_sb, et_ps)

                # out_h = attn @ vT -> ot[:, h, n] (single copy per tile)
                for n5 in range(N5):
                    op_ = hps.tile([CH, 512], F32, tag="hp")
                    nc.tensor.matmul(op_, et_sb, vt_sb[:, n5 * 512:(n5 + 1) * 512],
                                     start=True, stop=True)
                    nc.vector.tensor_copy(ot[:, h, n5 * 512:(n5 + 1) * 512], op_)

        # ---- Phase T fused with group-0 qk: x -> xT + q/k(g0), PE dense ----
        # x loads start before the 7MB Wqkv load so the transposes (and HAM
        # warmup) begin immediately; qk blocks trail by 3 iterations.
        LAG = KC + 1
        qk0 = qk_pool.tile([P, NB, 2 * GC], BF, tag="qk")
        nqk0 = nrmps.tile([1, 2 * GC], F32, tag="nqk")
        with tc.tile_pool(name="xstage", bufs=4) as xstage, \
             tc.tile_pool(name="wstage", bufs=2) as wstage, \
             tc.tile_pool(name="tps", bufs=2, space="PSUM") as tps:
            for nb in range(NB + LAG):
                if nb < NB:
                    xt_ = xstage.tile([P, C], F32, tag="x")
                    nc.sync.dma_start(out=xt_, in_=x_d[nb * P:(nb + 1) * P, :])
                    tall = tps.tile([P, KC, P], F32, tag="t")  # banks: kc 0-3|4-5
                    for kc in range(KC):
                        nc.tensor.matmul(tall[:, kc, :], xt_[:, kc * P:(kc + 1) * P],
                                         ident128, is_transpose=True,
                                         start=(kc in (0, 4)), stop=(kc in (3, 5)))
                    nc.vector.tensor_copy(xT[:, :, nb * P:(nb + 1) * P], tall)
                if 1 <= nb <= KC:
                    # one Wqkv chunk per iteration, interleaved with x loads so
                    # neither stream queues fully behind the other; the q|k
                    # half is cast separately so qk matmuls unblock early
                    kc = nb - 1
                    half = 3 * C // 2
                    st = wstage.tile([P, 3 * C], F32, tag="wst")
                    nc.sync.dma_start(out=st[:, 0:half],
                                      in_=wqkv_d[kc * P:(kc + 1) * P, 0:half])
                    nc.vector.tensor_copy(wq[:, kc, 0:half], st[:, 0:half])
                    nc.sync.dma_start(out=st[:, half:],
                                      in_=wqkv_d[kc * P:(kc + 1) * P, half:])
                    nc.vector.tensor_copy(wq[:, kc, half:], st[:, half:])
                if nb == KC + 1:
                    nc.sync.dma_start(out=temp_sb,
                                      in_=temp_d.rearrange("(a h) -> a h", a=1))
                    nc.sync.dma_start(out=bstage,
                                      in_=bproj_d.rearrange("(a c) -> a c", a=1))
                    nc.vector.tensor_copy(bstage_bf, bstage)
                if nb >= LAG:
                    qk_block(0, nb - LAG, qk0, nqk0)

        # heads pools open after the transpose PSUM pool closes (bank budget)
        vt_pool = gctx.enter_context(tc.tile_pool(name="vt", bufs=2))
        hps = gctx.enter_context(tc.tile_pool(name="hps", bufs=2, space="PSUM"))
        sps = gctx.enter_context(tc.tile_pool(name="sps", bufs=2, space="PSUM"))
        tinyps = gctx.enter_context(tc.tile_pool(name="tinyps", bufs=1, space="PSUM"))

        heads_phase(0, qk0, nqk0)
        for g in range(1, G):
            qk_sb = qk_pool.tile([P, NB, 2 * GC], BF, tag="qk")
            nqk = nrmps.tile([1, 2 * GC], F32, tag="nqk")
            for nb in range(NB):
                qk_block(g, nb, qk_sb, nqk)
            heads_phase(g, qk_sb, nqk)

        gctx.close()

        # ---- Phase PROJ: y = OT^T @ Wproj + bproj (8 K=96 chunks, head-major)
        with tc.tile_pool(name="wp", bufs=1) as wp_pool, \
             tc.tile_pool(name="wpstage", bufs=2) as wpstage, \
             tc.tile_pool(name="yout", bufs=3) as yout, \
             tc.tile_pool(name="yps", bufs=2, space="PSUM") as yps:
            wp = wp_pool.tile([CH, H, C], BF)   # wp[c, h, j] = Wproj[h*CH+c, j]
            for h in range(H):
                st = wpstage.tile([CH, C], F32, tag="wpst")
                nc.sync.dma_start(out=st, in_=wproj_d[h * CH:(h + 1) * CH, :])
                nc.vector.tensor_copy(wp[:, h, :], st)

            # bias broadcast to all 128 rows via K=1 matmul, once; then the
            # PROJ eviction is an add instead of a copy (no per-block bias MMs)
            bias_sb = wp_pool.tile([P, C], F32)
            for (a, b) in ((0, 512), (512, C)):
                bps = yps.tile([P, b - a], F32, tag="y1")
                nc.tensor.matmul(bps, ones_row, bstage_bf[0:1, a:b],
                                 start=True, stop=True)
                nc.vector.tensor_copy(bias_sb[:, a:b], bps)

            for nb in range(NB):
                y1 = yps.tile([P, 512], F32, tag="y1")
                y2 = yps.tile([P, 256], F32, tag="y2")
                for h in range(H):
                    lhsT = ot[:, h, nb * P:(nb + 1) * P]
                    nc.tensor.matmul(y1, lhsT, wp[:, h, 0:512],
                                     start=(h == 0), stop=(h == H - 1))
                    nc.tensor.matmul(y2, lhsT, wp[:, h, 512:C],
                                     start=(h == 0), stop=(h == H - 1))
                ysb = yout.tile([P, C], F32, tag="y")
                nc.vector.tensor_add(ysb[:, 0:512], y1, bias_sb[:, 0:512])
                nc.vector.tensor_add(ysb[:, 512:C], y2, bias_sb[:, 512:C])
                nc.sync.dma_start(out=y_d[nb * P:(nb + 1) * P, :], in_=ysb)

    # Split multi-wait sync conditions into EventSemaphore instructions —
    # walrus' ACT/DVE instruction structs encode at most one wait.
    bass_rust.generate_event_semaphores(nc)
    return nc


def _in_maps(x, Wqkv, temperature, Wproj, bproj):
    x = np.asarray(x)  # plain numpy before slicing (inputs may be jax arrays)
    wqkv = np.ascontiguousarray(Wqkv, dtype=np.float32)
    temp = np.ascontiguousarray(temperature, dtype=np.float32).reshape(H)
    wproj = np.ascontiguousarray(Wproj, dtype=np.float32)
    bp = np.ascontiguousarray(bproj, dtype=np.float32)
    return [
        {"x": np.ascontiguousarray(x[b], dtype=np.float32), "Wqkv": wqkv,
         "temperature": temp, "Wproj": wproj, "bproj": bp}
        for b in range(x.shape[0])
    ]


def run(x, Wqkv, temperature, Wproj, bproj, trace=False):
    nc = build_nc()
    in_maps = _in_maps(x, Wqkv, temperature, Wproj, bproj)
    res = run_bass_kernel_spmd(nc, in_maps, core_ids=list(range(len(in_maps))),
                               trace=trace)
    out = np.stack([res.results[b]["y"] for b in range(len(in_maps))], axis=0)
    return out.astype(np.float32), res


def kernel(x, Wqkv, temperature, Wproj, bproj):
    out, _ = run(x, Wqkv, temperature, Wproj, bproj, trace=False)
    return out


# revision 33
# speedup vs baseline: 1.3516x; 1.0070x over previous
"""Channel-attention (XCA) block on 8 trn2 NeuronCores, data-parallel over batch.

Per core: x (4096, 768) -> qkv -> per-head channel attention (96x96 scores over
l2-normalized q,k transposed to (Ch, N)) -> proj.  All big matmuls run in bf16
with fp32 PSUM accumulation; norms/softmax in fp32.

v3: group-0 qk generation fused into the x-transpose loop (PE dense from the
start, HAM stays warm), head-major attention-output layout, S-before-vT
software pipeline per head (softmax latency hidden under vT matmuls), single
eviction copies via 3D tiles.
"""

import numpy as np
from contextlib import ExitStack

import bass_rust
import concourse.bass as bass
import concourse.tile as tile
from concourse import mybir
from concourse.masks import make_identity
from concourse.bass_utils import run_bass_kernel_spmd

F32 = mybir.dt.float32
BF = mybir.dt.bfloat16
AF = mybir.ActivationFunctionType

P = 128          # partitions
N = 4096         # tokens per core (batch element)
C = 768          # channels
H = 8            # heads
CH = 96          # channels per head
KC = C // P      # 6 contraction chunks of 128
NB = N // P      # 32 token blocks of 128
N5 = N // 512    # 8 token blocks of 512
G = 4            # head groups
HPG = H // G     # 2 heads per group
GC = HPG * CH    # 192 qkv columns per group
EPS = 1e-12


def build_nc():
    nc = bass.Bass()

    x_d = nc.dram_tensor("x", [N, C], F32, kind="ExternalInput")
    wqkv_d = nc.dram_tensor("Wqkv", [C, 3 * C], F32, kind="ExternalInput")
    temp_d = nc.dram_tensor("temperature", [H], F32, kind="ExternalInput")
    wproj_d = nc.dram_tensor("Wproj", [C, C], F32, kind="ExternalInput")
    bproj_d = nc.dram_tensor("bproj", [C], F32, kind="ExternalInput")
    y_d = nc.dram_tensor("y", [N, C], F32, kind="ExternalOutput")

    with ExitStack() as ctx:
        tc = ctx.enter_context(tile.TileContext(nc))
        persist = ctx.enter_context(tc.tile_pool(name="persist", bufs=1))

        # persistent SBUF: xT[c%128, c//128, n] = x[n, c]  (bf16)
        xT = persist.tile([P, KC, N], BF)
        # Wqkv bf16: wq[c%128, c//128, j] = Wqkv[c, j]
        wq = persist.tile([P, KC, 3 * C], BF)
        # attention output, head-major: ot[c, h, n] = O[n, h*CH + c]
        ot = persist.tile([CH, H, N], BF)

        ident128 = persist.tile([P, P], F32)
        make_identity(nc, ident128)
        ident96 = persist.tile([CH, CH], BF)
        make_identity(nc, ident96)
        ones_col = persist.tile([P, 1], BF)      # norm-matmul lhsT (K=128, M=1)
        nc.vector.memset(ones_col, 1.0)
        ones_row = persist.tile([1, P], BF)      # bias-matmul lhsT (K=1, M=128)
        nc.vector.memset(ones_row, 1.0)
        one1 = persist.tile([1, 1], F32)         # row->col matmul rhs
        nc.vector.memset(one1, 1.0)
        ones96 = persist.tile([1, CH], F32)
        nc.vector.memset(ones96, 1.0)

        temp_sb = persist.tile([1, H], F32)
        bstage = persist.tile([1, C], F32)
        bstage_bf = persist.tile([1, C], BF)

        gctx = ctx.enter_context(ExitStack())
        qk_pool = gctx.enter_context(tc.tile_pool(name="qk", bufs=1))
        qkps = gctx.enter_context(tc.tile_pool(name="qkps", bufs=2, space="PSUM"))
        nrmps = gctx.enter_context(tc.tile_pool(name="nrmps", bufs=1, space="PSUM"))
        sqpool = gctx.enter_context(tc.tile_pool(name="sq", bufs=3))
        small = gctx.enter_context(tc.tile_pool(name="small", bufs=2))

        def qk_block(g, nb, qk_sb, nqk, pend):
            """qkv q|k matmuls for one token block + eviction + norm matmul.

            The norm matmul for block nb is emitted during block nb+1 (flushed
            by norm_flush after the loop) so the in-order PE never waits on the
            DVE-evict -> ACT-square chain that produces its operand."""
            qkp = qkps.tile([P, 2 * GC], F32, tag="qkp")
            qp = qkp[:, 0:GC]
            kp = qkp[:, GC:2 * GC]
            # qp|kp live in one PSUM bank: one accumulation group (start
            # pending-zeroes the whole bank, k region accumulates onto 0)
            for kc in range(KC):
                lhsT = xT[:, kc, nb * P:(nb + 1) * P]
                nc.tensor.matmul(
                    qp, lhsT, wq[:, kc, g * GC:(g + 1) * GC],
                    start=(kc == 0), stop=False)
                nc.tensor.matmul(
                    kp, lhsT, wq[:, kc, C + g * GC: C + (g + 1) * GC],
                    start=False, stop=(kc == KC - 1))
            if pend["sq"] is not None:
                nc.tensor.matmul(nqk, ones_col, pend["sq"],
                                 start=(pend["nb"] == 0), stop=False)
            qks = qk_sb[:, nb, :]
            nc.vector.tensor_copy(qks, qkp)
            sq = sqpool.tile([P, 2 * GC], BF, tag="sq")
            nc.scalar.activation(sq, qks, AF.Square)
            pend["sq"], pend["nb"] = sq, nb

        def norm_flush(nqk, pend):
            nc.tensor.matmul(nqk, ones_col, pend["sq"],
                             start=(pend["nb"] == 0), stop=True)
            pend["sq"] = None

        def heads_phase(g, qk_sb, nqk):
            """scores, softmax (hidden under vT gen), out tiles for one group."""
            # scores first: PE stays busy while the norm chain completes
            s_list = []
            for hh in range(HPG):
                s_ps = sps.tile([CH, CH], F32, tag="s")
                for nb in range(NB):
                    nc.tensor.matmul(
                        s_ps,
                        qk_sb[:, nb, hh * CH:(hh + 1) * CH],
                        qk_sb[:, nb, GC + hh * CH: GC + (hh + 1) * CH],
                        start=(nb == 0), stop=(nb == NB - 1))
                s_list.append(s_ps)

            # rinv row: 1 / max(sqrt(sumsq), eps), [q | k] in one (1, 384) row
            rqk = small.tile([1, 2 * GC], F32, tag="rqk")
            nc.scalar.activation(rqk, nqk, AF.Sqrt)
            nc.vector.tensor_scalar_max(rqk, rqk, EPS)
            nc.vector.reciprocal(rqk, rqk)

            # per-head norm-derived tiles (tiny PE matmuls, off the pipeline)
            rq_cols, r_sbs = [], []
            for hh in range(HPG):
                h = g * HPG + hh
                rq_ps = tinyps.tile([CH, 1], F32, tag="tp")
                nc.tensor.matmul(rq_ps, rqk[0:1, hh * CH:(hh + 1) * CH], one1,
                                 start=True, stop=True)
                rq_col = small.tile([CH, 1], F32, tag="rqc")
                nc.vector.tensor_copy(rq_col, rq_ps)
                tempb = small.tile([1, CH], F32, tag="tb")
                nc.scalar.activation(tempb, ones96, AF.Copy,
                                     scale=temp_sb[0:1, h:h + 1])
                r_ps = tinyps.tile([CH, CH], F32, tag="tp")
                nc.tensor.matmul(r_ps, tempb,
                                 rqk[0:1, GC + hh * CH: GC + (hh + 1) * CH],
                                 start=True, stop=True)
                r_sb = small.tile([CH, CH], F32, tag="rsb")
                nc.vector.tensor_copy(r_sb, r_ps)
                rq_cols.append(rq_col)
                r_sbs.append(r_sb)

            for hh in range(HPG):
                h = g * HPG + hh
                # softmax chain on DVE/ACT — overlaps the vT matmuls below
                z_sb = small.tile([CH, CH], F32, tag="z")
                nc.vector.tensor_mul(z_sb, s_list[hh], r_sbs[hh])
                e_sb = small.tile([CH, CH], BF, tag="e")
                sume = small.tile([CH, 1], F32, tag="se")
                nc.scalar.activation(e_sb, z_sb, AF.Exp,
                                     scale=rq_cols[hh], accum_out=sume)
                rden = small.tile([CH, 1], F32, tag="rd")
                nc.vector.reciprocal(rden, sume)
                attn_s = small.tile([CH, CH], BF, tag="at")
                nc.scalar.activation(attn_s, e_sb, AF.Copy, scale=rden)

                # vT for this head: (96 d, 4096 n) bf16, from Wqkv v-cols and xT
                vt_sb = vt_pool.tile([CH, N], BF, tag="vt")
                for n5 in range(N5):
                    vp = hps.tile([CH, 512], F32, tag="hp")
                    for kc in range(KC):
                        nc.tensor.matmul(
                            vp, wq[:, kc, 2 * C + h * CH: 2 * C + (h + 1) * CH],
                            xT[:, kc, n5 * 512:(n5 + 1) * 512],
                            start=(kc == 0), stop=(kc == KC - 1))
                    nc.vector.tensor_copy(vt_sb[:, n5 * 512:(n5 + 1) * 512], vp)

                # attn^T via PE transpose (ready well before vT finishes)
                et_ps = tinyps.tile([CH, CH], BF, tag="tp")
                nc.tensor.transpose(et_ps, attn_s, ident96)
                et_sb = small.tile([CH, CH], BF, tag="et")
                nc.vector.tensor_copy(et_sb, et_ps)

                # out_h = attn @ vT -> ot[:, h, n] (single copy per tile)
                for n5 in range(N5):
                    op_ = hps.tile([CH, 512], F32, tag="hp")
                    nc.tensor.matmul(op_, et_sb, vt_sb[:, n5 * 512:(n5 + 1) * 512],
                                     start=True, stop=True)
                    nc.vector.tensor_copy(ot[:, h, n5 * 512:(n5 + 1) * 512], op_)

        # ---- Phase T fused with group-0 qk: x -> xT + q/k(g0), PE dense ----
        # x loads start before the 7MB Wqkv load so the transposes (and HAM
        # warmup) begin immediately; qk blocks trail by 3 iterations.
        LAG = KC + 1
        qk0 = qk_pool.tile([P, NB, 2 * GC], BF, tag="qk")
        nqk0 = nrmps.tile([1, 2 * GC], F32, tag="nqk")
        pend0 = {"sq": None, "nb": -1}
        with tc.tile_pool(name="xstage", bufs=4) as xstage, \
             tc.tile_pool(name="wstage", bufs=2) as wstage, \
             tc.tile_pool(name="tps", bufs=2, space="PSUM") as tps:
            for nb in range(NB + LAG):
                if nb < NB:
                    xt_ = xstage.tile([P, C], F32, tag="x")
                    nc.sync.dma_start(out=xt_, in_=x_d[nb * P:(nb + 1) * P, :])
                    tall = tps.tile([P, KC, P], F32, tag="t")  # banks: kc 0-3|4-5
                    for kc in range(KC):
                        nc.tensor.matmul(tall[:, kc, :], xt_[:, kc * P:(kc + 1) * P],
                                         ident128, is_transpose=True,
                                         start=(kc in (0, 4)), stop=(kc in (3, 5)))
                    nc.vector.tensor_copy(xT[:, :, nb * P:(nb + 1) * P], tall)
                if 1 <= nb <= KC:
                    # one Wqkv chunk per iteration, interleaved with x loads so
                    # neither stream queues fully behind the other; the q|k
                    # half is cast separately so qk matmuls unblock early
                    kc = nb - 1
                    half = 3 * C // 2
                    st = wstage.tile([P, 3 * C], F32, tag="wst")
                    nc.sync.dma_start(out=st[:, 0:half],
                                      in_=wqkv_d[kc * P:(kc + 1) * P, 0:half])
                    nc.vector.tensor_copy(wq[:, kc, 0:half], st[:, 0:half])
                    nc.sync.dma_start(out=st[:, half:],
                                      in_=wqkv_d[kc * P:(kc + 1) * P, half:])
                    nc.vector.tensor_copy(wq[:, kc, half:], st[:, half:])
                if nb == KC + 1:
                    nc.sync.dma_start(out=temp_sb,
                                      in_=temp_d.rearrange("(a h) -> a h", a=1))
                    nc.sync.dma_start(out=bstage,
                                      in_=bproj_d.rearrange("(a c) -> a c", a=1))
                    nc.vector.tensor_copy(bstage_bf, bstage)
                if nb >= LAG:
                    qk_block(0, nb - LAG, qk0, nqk0, pend0)

        # heads pools open after the transpose PSUM pool closes (bank budget)
        vt_pool = gctx.enter_context(tc.tile_pool(name="vt", bufs=2))
        hps = gctx.enter_context(tc.tile_pool(name="hps", bufs=2, space="PSUM"))
        sps = gctx.enter_context(tc.tile_pool(name="sps", bufs=2, space="PSUM"))
        tinyps = gctx.enter_context(tc.tile_pool(name="tinyps", bufs=1, space="PSUM"))

        norm_flush(nqk0, pend0)
        heads_phase(0, qk0, nqk0)
        for g in range(1, G):
            qk_sb = qk_pool.tile([P, NB, 2 * GC], BF, tag="qk")
            nqk = nrmps.tile([1, 2 * GC], F32, tag="nqk")
            pend = {"sq": None, "nb": -1}
            for nb in range(NB):
                qk_block(g, nb, qk_sb, nqk, pend)
            norm_flush(nqk, pend)
            heads_phase(g, qk_sb, nqk)

        gctx.close()

        # ---- Phase PROJ: y = OT^T @ Wproj + bproj (8 K=96 chunks, head-major)
        with tc.tile_pool(name="wp", bufs=1) as wp_pool, \
             tc.tile_pool(name="wpstage", bufs=2) as wpstage, \
             tc.tile_pool(name="yout", bufs=3) as yout, \
             tc.tile_pool(name="yps", bufs=2, space="PSUM") as yps:
            wp = wp_pool.tile([CH, H, C], BF)   # wp[c, h, j] = Wproj[h*CH+c, j]
            for h in range(H):
                st = wpstage.tile([CH, C], F32, tag="wpst")
                nc.sync.dma_start(out=st, in_=wproj_d[h * CH:(h + 1) * CH, :])
                nc.vector.tensor_copy(wp[:, h, :], st)

            # bias broadcast to all 128 rows via K=1 matmul, once; then the
            # PROJ eviction is an add instead of a copy (no per-block bias MMs)
            bias_sb = wp_pool.tile([P, C], F32)
            for (a, b) in ((0, 512), (512, C)):
                bps = yps.tile([P, b - a], F32, tag="y1")
                nc.tensor.matmul(bps, ones_row, bstage_bf[0:1, a:b],
                                 start=True, stop=True)
                nc.vector.tensor_copy(bias_sb[:, a:b], bps)

            for nb in range(NB):
                y1 = yps.tile([P, 512], F32, tag="y1")
                y2 = yps.tile([P, 256], F32, tag="y2")
                for h in range(H):
                    lhsT = ot[:, h, nb * P:(nb + 1) * P]
                    nc.tensor.matmul(y1, lhsT, wp[:, h, 0:512],
                                     start=(h == 0), stop=(h == H - 1))
                    nc.tensor.matmul(y2, lhsT, wp[:, h, 512:C],
                                     start=(h == 0), stop=(h == H - 1))
                ysb = yout.tile([P, C], F32, tag="y")
                nc.vector.tensor_add(ysb[:, 0:512], y1, bias_sb[:, 0:512])
                nc.vector.tensor_add(ysb[:, 512:C], y2, bias_sb[:, 512:C])
                nc.sync.dma_start(out=y_d[nb * P:(nb + 1) * P, :], in_=ysb)

    # Split multi-wait sync conditions into EventSemaphore instructions —
    # walrus' ACT/DVE instruction structs encode at most one wait.
    bass_rust.generate_event_semaphores(nc)
    return nc


def _in_maps(x, Wqkv, temperature, Wproj, bproj):
    x = np.asarray(x)  # plain numpy before slicing (inputs may be jax arrays)
    wqkv = np.ascontiguousarray(Wqkv, dtype=np.float32)
    temp = np.ascontiguousarray(temperature, dtype=np.float32).reshape(H)
    wproj = np.ascontiguousarray(Wproj, dtype=np.float32)
    bp = np.ascontiguousarray(bproj, dtype=np.float32)
    return [
        {"x": np.ascontiguousarray(x[b], dtype=np.float32), "Wqkv": wqkv,
         "temperature": temp, "Wproj": wproj, "bproj": bp}
        for b in range(x.shape[0])
    ]


def run(x, Wqkv, temperature, Wproj, bproj, trace=False):
    nc = build_nc()
    in_maps = _in_maps(x, Wqkv, temperature, Wproj, bproj)
    res = run_bass_kernel_spmd(nc, in_maps, core_ids=list(range(len(in_maps))),
                               trace=trace)
    out = np.stack([res.results[b]["y"] for b in range(len(in_maps))], axis=0)
    return out.astype(np.float32), res


def kernel(x, Wqkv, temperature, Wproj, bproj):
    out, _ = run(x, Wqkv, temperature, Wproj, bproj, trace=False)
    return out


# revision 34
# speedup vs baseline: 1.3683x; 1.0123x over previous
"""Channel-attention (XCA) block on 8 trn2 NeuronCores, data-parallel over batch.

Per core: x (4096, 768) -> qkv -> per-head channel attention (96x96 scores over
l2-normalized q,k transposed to (Ch, N)) -> proj.  All big matmuls run in bf16
with fp32 PSUM accumulation; norms/softmax in fp32.

v3: group-0 qk generation fused into the x-transpose loop (PE dense from the
start, HAM stays warm), head-major attention-output layout, S-before-vT
software pipeline per head (softmax latency hidden under vT matmuls), single
eviction copies via 3D tiles.
"""

import numpy as np
from contextlib import ExitStack

import bass_rust
import concourse.bass as bass
import concourse.tile as tile
from concourse import mybir
from concourse.masks import make_identity
from concourse.bass_utils import run_bass_kernel_spmd

F32 = mybir.dt.float32
BF = mybir.dt.bfloat16
AF = mybir.ActivationFunctionType

P = 128          # partitions
N = 4096         # tokens per core (batch element)
C = 768          # channels
H = 8            # heads
CH = 96          # channels per head
KC = C // P      # 6 contraction chunks of 128
NB = N // P      # 32 token blocks of 128
N5 = N // 512    # 8 token blocks of 512
G = 4            # head groups
HPG = H // G     # 2 heads per group
GC = HPG * CH    # 192 qkv columns per group
EPS = 1e-12


def build_nc():
    nc = bass.Bass()

    x_d = nc.dram_tensor("x", [N, C], F32, kind="ExternalInput")
    wqkv_d = nc.dram_tensor("Wqkv", [C, 3 * C], F32, kind="ExternalInput")
    temp_d = nc.dram_tensor("temperature", [H], F32, kind="ExternalInput")
    wproj_d = nc.dram_tensor("Wproj", [C, C], F32, kind="ExternalInput")
    bproj_d = nc.dram_tensor("bproj", [C], F32, kind="ExternalInput")
    y_d = nc.dram_tensor("y", [N, C], F32, kind="ExternalOutput")

    with ExitStack() as ctx:
        tc = ctx.enter_context(tile.TileContext(nc))
        persist = ctx.enter_context(tc.tile_pool(name="persist", bufs=1))

        # persistent SBUF: xT[c%128, c//128, n] = x[n, c]  (bf16)
        xT = persist.tile([P, KC, N], BF)
        # Wqkv bf16: wq[c%128, c//128, j] = Wqkv[c, j]
        wq = persist.tile([P, KC, 3 * C], BF)
        # attention output, head-major: ot[c, h, n] = O[n, h*CH + c]
        ot = persist.tile([CH, H, N], BF)

        ident128 = persist.tile([P, P], F32)
        make_identity(nc, ident128)
        ident96 = persist.tile([CH, CH], BF)
        make_identity(nc, ident96)
        ones_col = persist.tile([P, 1], BF)      # norm-matmul lhsT (K=128, M=1)
        nc.vector.memset(ones_col, 1.0)
        ones_row = persist.tile([1, P], BF)      # bias-matmul lhsT (K=1, M=128)
        nc.vector.memset(ones_row, 1.0)
        one1 = persist.tile([1, 1], F32)         # row->col matmul rhs
        nc.vector.memset(one1, 1.0)
        ones96 = persist.tile([1, CH], F32)
        nc.vector.memset(ones96, 1.0)

        temp_sb = persist.tile([1, H], F32)
        bstage = persist.tile([1, C], F32)
        bstage_bf = persist.tile([1, C], BF)

        gctx = ctx.enter_context(ExitStack())
        qk_pool = gctx.enter_context(tc.tile_pool(name="qk", bufs=1))
        qkps = gctx.enter_context(tc.tile_pool(name="qkps", bufs=2, space="PSUM"))
        nrmps = gctx.enter_context(tc.tile_pool(name="nrmps", bufs=1, space="PSUM"))
        sqpool = gctx.enter_context(tc.tile_pool(name="sq", bufs=3))
        small = gctx.enter_context(tc.tile_pool(name="small", bufs=2))

        def qk_block(g, nb, qk_sb, nqk, pend):
            """qkv q|k matmuls for one token block + eviction + norm matmul.

            The norm matmul for block nb is emitted during block nb+1 (flushed
            by norm_flush after the loop) so the in-order PE never waits on the
            DVE-evict -> ACT-square chain that produces its operand."""
            qkp = qkps.tile([P, 2 * GC], F32, tag="qkp")
            qp = qkp[:, 0:GC]
            kp = qkp[:, GC:2 * GC]
            # qp|kp live in one PSUM bank: one accumulation group (start
            # pending-zeroes the whole bank, k region accumulates onto 0)
            for kc in range(KC):
                lhsT = xT[:, kc, nb * P:(nb + 1) * P]
                nc.tensor.matmul(
                    qp, lhsT, wq[:, kc, g * GC:(g + 1) * GC],
                    start=(kc == 0), stop=False)
                nc.tensor.matmul(
                    kp, lhsT, wq[:, kc, C + g * GC: C + (g + 1) * GC],
                    start=False, stop=(kc == KC - 1))
            if pend["sq"] is not None:
                nc.tensor.matmul(nqk, ones_col, pend["sq"],
                                 start=(pend["nb"] == 0), stop=False)
            qks = qk_sb[:, nb, :]
            nc.vector.tensor_copy(qks, qkp)
            sq = sqpool.tile([P, 2 * GC], BF, tag="sq")
            nc.scalar.activation(sq, qks, AF.Square)
            pend["sq"], pend["nb"] = sq, nb

        def norm_flush(nqk, pend):
            nc.tensor.matmul(nqk, ones_col, pend["sq"],
                             start=(pend["nb"] == 0), stop=True)
            pend["sq"] = None

        def heads_phase(g, qk_sb, nqk):
            """scores, softmax (hidden under vT gen), out tiles for one group."""
            # scores first: PE stays busy while the norm chain completes
            s_list = []
            for hh in range(HPG):
                s_ps = sps.tile([CH, CH], F32, tag="s")
                for nb in range(NB):
                    nc.tensor.matmul(
                        s_ps,
                        qk_sb[:, nb, hh * CH:(hh + 1) * CH],
                        qk_sb[:, nb, GC + hh * CH: GC + (hh + 1) * CH],
                        start=(nb == 0), stop=(nb == NB - 1))
                s_list.append(s_ps)

            # rinv row: 1 / max(sqrt(sumsq), eps), [q | k] in one (1, 384) row
            rqk = small.tile([1, 2 * GC], F32, tag="rqk")
            nc.scalar.activation(rqk, nqk, AF.Sqrt)
            nc.vector.tensor_scalar_max(rqk, rqk, EPS)
            nc.vector.reciprocal(rqk, rqk)

            # per-head norm-derived tiles (tiny PE matmuls, off the pipeline)
            rq_cols, r_sbs = [], []
            for hh in range(HPG):
                h = g * HPG + hh
                rq_ps = tinyps.tile([CH, 1], F32, tag="tp")
                nc.tensor.matmul(rq_ps, rqk[0:1, hh * CH:(hh + 1) * CH], one1,
                                 start=True, stop=True)
                rq_col = small.tile([CH, 1], F32, tag="rqc")
                nc.vector.tensor_copy(rq_col, rq_ps)
                tempb = small.tile([1, CH], F32, tag="tb")
                nc.scalar.activation(tempb, ones96, AF.Copy,
                                     scale=temp_sb[0:1, h:h + 1])
                r_ps = tinyps.tile([CH, CH], F32, tag="tp")
                nc.tensor.matmul(r_ps, tempb,
                                 rqk[0:1, GC + hh * CH: GC + (hh + 1) * CH],
                                 start=True, stop=True)
                r_sb = small.tile([CH, CH], F32, tag="rsb")
                nc.vector.tensor_copy(r_sb, r_ps)
                rq_cols.append(rq_col)
                r_sbs.append(r_sb)

            for hh in range(HPG):
                h = g * HPG + hh
                # softmax chain on DVE/ACT — overlaps the vT matmuls below
                z_sb = small.tile([CH, CH], F32, tag="z")
                nc.vector.tensor_mul(z_sb, s_list[hh], r_sbs[hh])
                e_sb = small.tile([CH, CH], BF, tag="e")
                sume = small.tile([CH, 1], F32, tag="se")
                nc.scalar.activation(e_sb, z_sb, AF.Exp,
                                     scale=rq_cols[hh], accum_out=sume)
                rden = small.tile([CH, 1], F32, tag="rd")
                nc.vector.reciprocal(rden, sume)
                attn_s = small.tile([CH, CH], BF, tag="at")
                nc.scalar.activation(attn_s, e_sb, AF.Copy, scale=rden)

                # vT for this head: (96 d, 4096 n) bf16, from Wqkv v-cols and xT
                vt_sb = vt_pool.tile([CH, N], BF, tag="vt")
                for n5 in range(N5):
                    vp = hps.tile([CH, 512], F32, tag="hp")
                    for kc in range(KC):
                        nc.tensor.matmul(
                            vp, wq[:, kc, 2 * C + h * CH: 2 * C + (h + 1) * CH],
                            xT[:, kc, n5 * 512:(n5 + 1) * 512],
                            start=(kc == 0), stop=(kc == KC - 1))
                    nc.vector.tensor_copy(vt_sb[:, n5 * 512:(n5 + 1) * 512], vp)

                # attn^T via PE transpose (ready well before vT finishes)
                et_ps = tinyps.tile([CH, CH], BF, tag="tp")
                nc.tensor.transpose(et_ps, attn_s, ident96)
                et_sb = small.tile([CH, CH], BF, tag="et")
                nc.vector.tensor_copy(et_sb, et_ps)

                # out_h = attn @ vT -> ot[:, h, n]; evictions alternate DVE/ACT
                # so PSUM slots recycle fast enough to feed the next head's vT
                for n5 in range(N5):
                    op_ = hps.tile([CH, 512], F32, tag="hp")
                    nc.tensor.matmul(op_, et_sb, vt_sb[:, n5 * 512:(n5 + 1) * 512],
                                     start=True, stop=True)
                    dst = ot[:, h, n5 * 512:(n5 + 1) * 512]
                    if n5 % 2 == 0:
                        nc.vector.tensor_copy(dst, op_)
                    else:
                        nc.scalar.activation(dst, op_, AF.Copy)

        # ---- Phase T fused with group-0 qk: x -> xT + q/k(g0), PE dense ----
        # x loads start before the 7MB Wqkv load so the transposes (and HAM
        # warmup) begin immediately; qk blocks trail by 3 iterations.
        LAG = KC + 1
        qk0 = qk_pool.tile([P, NB, 2 * GC], BF, tag="qk")
        nqk0 = nrmps.tile([1, 2 * GC], F32, tag="nqk")
        pend0 = {"sq": None, "nb": -1}
        with tc.tile_pool(name="xstage", bufs=4) as xstage, \
             tc.tile_pool(name="wstage", bufs=2) as wstage, \
             tc.tile_pool(name="tps", bufs=2, space="PSUM") as tps:
            for nb in range(NB + LAG):
                if nb < NB:
                    xt_ = xstage.tile([P, C], F32, tag="x")
                    nc.sync.dma_start(out=xt_, in_=x_d[nb * P:(nb + 1) * P, :])
                    tall = tps.tile([P, KC, P], F32, tag="t")  # banks: kc 0-3|4-5
                    for kc in range(KC):
                        nc.tensor.matmul(tall[:, kc, :], xt_[:, kc * P:(kc + 1) * P],
                                         ident128, is_transpose=True,
                                         start=(kc in (0, 4)), stop=(kc in (3, 5)))
                    nc.vector.tensor_copy(xT[:, :, nb * P:(nb + 1) * P], tall)
                if 1 <= nb <= KC:
                    # one Wqkv chunk per iteration, interleaved with x loads so
                    # neither stream queues fully behind the other; the q|k
                    # half is cast separately so qk matmuls unblock early
                    kc = nb - 1
                    half = 3 * C // 2
                    st = wstage.tile([P, 3 * C], F32, tag="wst")
                    nc.sync.dma_start(out=st[:, 0:half],
                                      in_=wqkv_d[kc * P:(kc + 1) * P, 0:half])
                    nc.vector.tensor_copy(wq[:, kc, 0:half], st[:, 0:half])
                    nc.sync.dma_start(out=st[:, half:],
                                      in_=wqkv_d[kc * P:(kc + 1) * P, half:])
                    nc.vector.tensor_copy(wq[:, kc, half:], st[:, half:])
                if nb == KC + 1:
                    nc.sync.dma_start(out=temp_sb,
                                      in_=temp_d.rearrange("(a h) -> a h", a=1))
                    nc.sync.dma_start(out=bstage,
                                      in_=bproj_d.rearrange("(a c) -> a c", a=1))
                    nc.vector.tensor_copy(bstage_bf, bstage)
                if nb >= LAG:
                    qk_block(0, nb - LAG, qk0, nqk0, pend0)

        # heads pools open after the transpose PSUM pool closes (bank budget)
        vt_pool = gctx.enter_context(tc.tile_pool(name="vt", bufs=2))
        hps = gctx.enter_context(tc.tile_pool(name="hps", bufs=2, space="PSUM"))
        sps = gctx.enter_context(tc.tile_pool(name="sps", bufs=2, space="PSUM"))
        tinyps = gctx.enter_context(tc.tile_pool(name="tinyps", bufs=1, space="PSUM"))

        norm_flush(nqk0, pend0)
        heads_phase(0, qk0, nqk0)
        for g in range(1, G):
            qk_sb = qk_pool.tile([P, NB, 2 * GC], BF, tag="qk")
            nqk = nrmps.tile([1, 2 * GC], F32, tag="nqk")
            pend = {"sq": None, "nb": -1}
            for nb in range(NB):
                qk_block(g, nb, qk_sb, nqk, pend)
            norm_flush(nqk, pend)
            heads_phase(g, qk_sb, nqk)

        gctx.close()

        # ---- Phase PROJ: y = OT^T @ Wproj + bproj (8 K=96 chunks, head-major)
        with tc.tile_pool(name="wp", bufs=1) as wp_pool, \
             tc.tile_pool(name="wpstage", bufs=2) as wpstage, \
             tc.tile_pool(name="yout", bufs=3) as yout, \
             tc.tile_pool(name="yps", bufs=2, space="PSUM") as yps:
            wp = wp_pool.tile([CH, H, C], BF)   # wp[c, h, j] = Wproj[h*CH+c, j]
            for h in range(H):
                st = wpstage.tile([CH, C], F32, tag="wpst")
                nc.sync.dma_start(out=st, in_=wproj_d[h * CH:(h + 1) * CH, :])
                nc.vector.tensor_copy(wp[:, h, :], st)

            # bias broadcast to all 128 rows via K=1 matmul, once; then the
            # PROJ eviction is an add instead of a copy (no per-block bias MMs)
            bias_sb = wp_pool.tile([P, C], F32)
            for (a, b) in ((0, 512), (512, C)):
                bps = yps.tile([P, b - a], F32, tag="y1")
                nc.tensor.matmul(bps, ones_row, bstage_bf[0:1, a:b],
                                 start=True, stop=True)
                nc.vector.tensor_copy(bias_sb[:, a:b], bps)

            for nb in range(NB):
                y1 = yps.tile([P, 512], F32, tag="y1")
                y2 = yps.tile([P, 256], F32, tag="y2")
                for h in range(H):
                    lhsT = ot[:, h, nb * P:(nb + 1) * P]
                    nc.tensor.matmul(y1, lhsT, wp[:, h, 0:512],
                                     start=(h == 0), stop=(h == H - 1))
                    nc.tensor.matmul(y2, lhsT, wp[:, h, 512:C],
                                     start=(h == 0), stop=(h == H - 1))
                ysb = yout.tile([P, C], F32, tag="y")
                nc.vector.tensor_add(ysb[:, 0:512], y1, bias_sb[:, 0:512])
                nc.vector.tensor_add(ysb[:, 512:C], y2, bias_sb[:, 512:C])
                nc.sync.dma_start(out=y_d[nb * P:(nb + 1) * P, :], in_=ysb)

    # Split multi-wait sync conditions into EventSemaphore instructions —
    # walrus' ACT/DVE instruction structs encode at most one wait.
    bass_rust.generate_event_semaphores(nc)
    return nc


def _in_maps(x, Wqkv, temperature, Wproj, bproj):
    x = np.asarray(x)  # plain numpy before slicing (inputs may be jax arrays)
    wqkv = np.ascontiguousarray(Wqkv, dtype=np.float32)
    temp = np.ascontiguousarray(temperature, dtype=np.float32).reshape(H)
    wproj = np.ascontiguousarray(Wproj, dtype=np.float32)
    bp = np.ascontiguousarray(bproj, dtype=np.float32)
    return [
        {"x": np.ascontiguousarray(x[b], dtype=np.float32), "Wqkv": wqkv,
         "temperature": temp, "Wproj": wproj, "bproj": bp}
        for b in range(x.shape[0])
    ]


def run(x, Wqkv, temperature, Wproj, bproj, trace=False):
    nc = build_nc()
    in_maps = _in_maps(x, Wqkv, temperature, Wproj, bproj)
    res = run_bass_kernel_spmd(nc, in_maps, core_ids=list(range(len(in_maps))),
                               trace=trace)
    out = np.stack([res.results[b]["y"] for b in range(len(in_maps))], axis=0)
    return out.astype(np.float32), res


def kernel(x, Wqkv, temperature, Wproj, bproj):
    out, _ = run(x, Wqkv, temperature, Wproj, bproj, trace=False)
    return out
